# revision 31
# baseline (speedup 1.0000x reference)
"""BitNet MLP (act_quant -> ternary matmul -> relu^2 -> SubLN -> act_quant ->
ternary matmul) on 8 Trainium2 NeuronCores, data-parallel over tokens.

Math notes (exactness):
- act_quant int levels (|q| <= 127) and ternary weights {-1,0,1} are exactly
  representable in bf16, so both matmuls run on the PE in bf16 with exact
  integer arithmetic (f32 PSUM accumulation, |sums| < 2^24).
- All quantization scales are folded into per-token scalars applied to the
  final [tok, 512] output: out = i2 * beta_t with
    beta_t = clip(c_t * alpha_t * Sabs_t, 1e-5) * clip(mean|w_dn|,1e-5) / 127
  where alpha_t = (clip(max|x_t|,1e-5) * clip(mean|w_up|,1e-5) / 127)^2,
  Sabs_t = max_i |relu(ih)^2 * g|, c_t = rsqrt(var_t + 1e-6).
- Rounding uses the magic-number trick (x + 1.5*2^23 - 1.5*2^23) == RNE
  round-to-integer for |x| < 2^22, matching jnp.round (half-to-even).
- SubLN variance is recovered from the quantized intermediate:
  var = alpha^2 * sum(iu^2) * (Sabs/127)^2 / (2048 * g0^2); the
  quantization error on sum(iu^2) is ~0.1% which is far below tolerance.
  (For non-constant g an extra pass computes sum((relu^2)^2) directly.)
"""
import os
import numpy as np

import concourse.bass as bass
import concourse.tile as tile
from concourse import mybir
from concourse.bass_utils import run_bass_kernel_spmd
from concourse.masks import make_identity

# ---------------------------------------------------------------------------
# Workaround for walrus "Too many sync wait commands" on the TileContext tail
# drain: split the drain's semaphore waits across single-wait SP NOPs, then
# advance the observed clocks so the real drain needs none.
import re as _re
import bass_rust as _bass_rust


def _patched_drain_and_barrier(self, tick_clock, wait_clock):
    gc = tick_clock.global_clock
    ticks = list(map(int, _re.findall(r"\d+", repr(gc))))
    n = len(ticks)
    nonzero = [(i, t) for i, t in enumerate(ticks) if t > 0]
    for i, t in nonzero:
        sub = [0] * n
        sub[i] = t
        sub_scoped = _bass_rust.ScopedClock({None: _bass_rust.VectorClock(sub)})
        nop = self.nc.sync.nop()
        wait_clock.add_sem_waits(nop.ins, sub_scoped)
        for ec in wait_clock.engine_clocks:
            ec.update_past(sub_scoped)
    drain_inst = self.nc.sync.drain()
    wait_clock.add_sem_waits(drain_inst.ins,
                             _bass_rust.ScopedClock({None: gc}))
    self.nc.all_engine_barrier()
    popped = self.nc._tile_sem_poison_stack.pop()
    assert popped is self._sem_poison
    self.nc.clear_and_free_semaphores(list(self.sems.allocated().values()))
    self.nc.all_engine_barrier()


tile.TileContext._drain_and_barrier = _patched_drain_and_barrier


def _split_sync_waits(nc, keep_default=1):
    """walrus caps the number of semaphore waits a single instruction can
    carry (CTRL ops take only 1; compute ops a few). Hoist excess waits onto
    single-wait NOPs inserted immediately before the instruction on the same
    engine — identical semantics, engines execute in order."""
    import dataclasses
    keep_by_op = {}
    proto = None
    for f in nc.m.functions:
        for bb in f.blocks:
            for inst in bb.instructions:
                if type(inst).__name__ == "InstNoOp":
                    proto = inst
                    break
            if proto is not None:
                break
        if proto is not None:
            break
    counter = [0]
    for f in nc.m.functions:
        new_blocks = []
        for bb in f.blocks:
            out = []
            changed = False
            for inst in bb.instructions:
                si = inst.sync_info
                ow = list(si.on_wait) if si is not None and si.on_wait else []
                keep = keep_by_op.get(inst.opcode, keep_default)
                if len(ow) > keep:
                    assert proto is not None, "no NoOp prototype found yet"
                    for w in ow[:-keep]:
                        counter[0] += 1
                        nop = dataclasses.replace(
                            proto,
                            name=f"I-waitsplit-{counter[0]}",
                            engine=inst.engine,
                            sync_info=_bass_rust.SyncInfo(on_wait=[w],
                                                          on_update=[]),
                        )
                        out.append(nop)
                    si.on_wait = ow[-keep:]
                    changed = True
                out.append(inst)
            if changed:
                bb2 = _bass_rust.BasicBlock(name=bb.name, instructions=out)
                bb2.IsExit = bb.IsExit
                bb2.IsLoopEntry = bb.IsLoopEntry
                bb2.IsPredicated = bb.IsPredicated
                new_blocks.append(bb2)
            else:
                new_blocks.append(bb)
        f.blocks = new_blocks
# ---------------------------------------------------------------------------

F32 = mybir.dt.float32
BF16 = mybir.dt.bfloat16
ALU = mybir.AluOpType
AF = mybir.ActivationFunctionType

N_CORES = 8
B, S, H, I = 8, 8192, 512, 2048
TOK = B * S                  # 65536 tokens total
TPC = TOK // N_CORES         # 8192 tokens per core
P = 128                      # partition tile
NT = TPC // P                # 64 token tiles per core
NKH = H // P                 # 4 k-tiles over H
NKI = I // P                 # 16 k-tiles over I
NB = I // 512                # 4 psum banks for the up matmul

MAGIC = 12582912.0           # 1.5 * 2^23: RNE round-to-int trick
EPS = 1e-6                   # SubLN eps (from reference)

LAST_RESULT = None           # set by kernel() for test harness introspection


# ===========================================================================
# Optimized build (constant g, the case the harness exercises)
# ===========================================================================

_DMA_QUEUES = None


def _dma_queue(nc, i):
    """Round-robin weight-prep DMAs across engine queues so the transfers
    overlap instead of serializing on one ring."""
    return [nc.gpsimd, nc.scalar, nc.sync][i % 3]


def _prep_weight_pass1(nc, stage, wT_dram, n_ktiles, width, name, consts):
    """DMA the weight tiles and accumulate per-partition |w| sums (DVE)."""
    asum = consts.tile([P, n_ktiles], F32, tag=f"{name}_asum")
    for k in range(n_ktiles):
        wf = stage.tile([P, width], F32, tag="wst")
        _dma_queue(nc, k).dma_start(out=wf, in_=wT_dram[k * P:(k + 1) * P, :])
        nc.vector.tensor_reduce(out=asum[:, k:k + 1], in_=wf,
                                axis=mybir.AxisListType.X, op=ALU.add,
                                apply_absolute_value=True)
    return asum


def _prep_weight_scale(nc, stage, ps, consts, asum, n_elem, name):
    """Reduce per-partition sums to the global clip(mean|w|,1e-5) broadcast
    to all partitions, plus its reciprocal."""
    tot = consts.tile([P, 1], F32, tag=f"{name}_tot")
    nc.vector.tensor_reduce(out=tot, in_=asum, axis=mybir.AxisListType.X,
                            op=ALU.add)
    ones128 = stage.tile([P, P], F32, tag=f"{name}_ones")
    nc.vector.memset(ones128, 1.0)
    totp = ps.tile([P, 1], F32, tag=f"{name}_totp")
    nc.tensor.matmul(out=totp, lhsT=ones128, rhs=tot, start=True, stop=True)
    gsum = consts.tile([P, 1], F32, tag=f"{name}_gsum")
    nc.scalar.copy(out=gsum, in_=totp)
    meanclip = consts.tile([P, 1], F32, tag=f"{name}_meanclip")
    nc.vector.tensor_scalar(out=meanclip, in0=gsum, scalar1=1.0 / n_elem,
                            scalar2=1e-5, op0=ALU.mult, op1=ALU.max)
    swq = consts.tile([P, 1], F32, tag=f"{name}_swq")
    nc.vector.reciprocal(out=swq, in_=meanclip)
    return meanclip, swq


def _prep_weight_pass2_tile(nc, stage, consts, wT_dram, k, width, name,
                            magicb, swq, out_dt):
    """Re-DMA one k-tile and quantize to ternary in dtype out_dt.
    Returns a [P, width] tile."""
    wf = stage.tile([P, width], F32, tag="wst")
    _dma_queue(nc, k).dma_start(out=wf, in_=wT_dram[k * P:(k + 1) * P, :])
    rt = stage.tile([P, width], F32, tag="wrt")
    nc.scalar.activation(out=rt, in_=wf, func=AF.Identity,
                         bias=magicb, scale=swq)
    nc.vector.tensor_scalar(out=rt, in0=rt, scalar1=MAGIC, scalar2=1.0,
                            op0=ALU.subtract, op1=ALU.min)
    wq = consts.tile([P, width], out_dt, tag=f"{name}_wq{k}")
    nc.vector.tensor_scalar(out=wq, in0=rt, scalar1=-1.0, scalar2=None,
                            op0=ALU.max)
    return wq


def build_nc_const():
    nc = bass.Bass()
    x_d = nc.dram_tensor("x", [TPC, H], F32, kind="ExternalInput")
    wupT_d = nc.dram_tensor("wupT", [H, I], F32, kind="ExternalInput")
    wdnT_d = nc.dram_tensor("wdnT", [I, H], F32, kind="ExternalInput")
    g_d = nc.dram_tensor("g", [I], F32, kind="ExternalInput")
    out_d = nc.dram_tensor("out", [TPC, H], F32, kind="ExternalOutput")

    BG = 8                    # tiles per small-op batch
    KV = 1.0 / (127.0 * 127.0 * I)

    from contextlib import ExitStack
    with ExitStack() as ctx:
        tc = ctx.enter_context(tile.TileContext(nc))

        consts = ctx.enter_context(tc.tile_pool(name="consts", bufs=1))

        ident = consts.tile([P, P], BF16)
        make_identity(nc, ident)
        magicb = consts.tile([P, 1], F32)
        nc.vector.memset(magicb, MAGIC)
        negmb = consts.tile([P, 1], F32)
        nc.vector.memset(negmb, -MAGIC)

        # -------- main-loop pools (created before prep so x prefetch can
        # land while weights stream in) --------
        xs_pool = ctx.enter_context(tc.tile_pool(name="xs", bufs=2 * BG))
        xq_pool = ctx.enter_context(tc.tile_pool(name="xqp", bufs=2))
        hlp = ctx.enter_context(tc.tile_pool(name="hlp", bufs=2))
        xTp = ctx.enter_context(tc.tile_pool(name="xTp", bufs=2))
        rp = ctx.enter_context(tc.tile_pool(name="rp", bufs=2))
        rtp = ctx.enter_context(tc.tile_pool(name="rtp", bufs=2))
        iup = ctx.enter_context(tc.tile_pool(name="iup", bufs=2))
        iuTp = ctx.enter_context(tc.tile_pool(name="iuTp", bufs=2))
        junkp = ctx.enter_context(tc.tile_pool(name="junkp", bufs=1))
        smallp = ctx.enter_context(tc.tile_pool(name="smallp", bufs=4))
        outp = ctx.enter_context(tc.tile_pool(name="outp", bufs=BG + 1))
        o2p = ctx.enter_context(tc.tile_pool(name="o2p", bufs=3))
        batchp = ctx.enter_context(tc.tile_pool(name="batchp", bufs=2))

        # -------- x prefetch state --------
        state = {}

        def phase_a(ib):
            """DMA x tiles + per-token absmax, then batched x-scale chain."""
            xm8 = batchp.tile([P, BG], F32, tag="xm8")
            x_tiles = []
            for j in range(BG):
                r0 = (ib + j) * P
                x_sb = xs_pool.tile([P, H], F32, tag="x")
                nc.sync.dma_start(out=x_sb, in_=x_d[r0:r0 + P, :])
                x_tiles.append(x_sb)
                nc.vector.tensor_reduce(out=xm8[:, j:j + 1], in_=x_sb,
                                        axis=mybir.AxisListType.X, op=ALU.max,
                                        apply_absolute_value=True)
            t08 = batchp.tile([P, BG], F32, tag="t08")
            nc.vector.tensor_scalar_max(out=t08, in0=xm8, scalar1=1e-5)
            xr8 = batchp.tile([P, BG], F32, tag="xr8")
            nc.vector.reciprocal(out=xr8, in_=t08)
            xsc8 = batchp.tile([P, BG], F32, tag="xsc8")
            nc.vector.tensor_scalar_mul(out=xsc8, in0=xr8, scalar1=127.0)
            state[ib] = (x_tiles, t08, xsc8)

        # issue the first two batches of x loads before weight prep so the
        # DMA and absmax work overlaps the weight streaming
        phase_a(0)
        phase_a(BG)

        # -------- weight prep --------
        with tc.tile_pool(name="wstage", bufs=2) as stage, \
                tc.tile_pool(name="wps", bufs=1, space="PSUM") as wps:
            # g0 broadcast [128,1] via K=1 matmul with ones (g is constant)
            ones_row = stage.tile([1, P], F32, tag="ones_row")
            nc.vector.memset(ones_row, 1.0)
            g0_sb = stage.tile([1, 1], F32, tag="g0sb")
            nc.gpsimd.dma_start(out=g0_sb, in_=g_d[0:1])
            g0_ps = wps.tile([P, 1], F32, tag="g0ps")
            nc.tensor.matmul(out=g0_ps, lhsT=ones_row, rhs=g0_sb, start=True,
                             stop=True)
            g0b = consts.tile([P, 1], F32)
            nc.scalar.copy(out=g0b, in_=g0_ps)

            up_asum = _prep_weight_pass1(nc, stage, wupT_d, NKH, I, "wup",
                                         consts)
            dn_asum = _prep_weight_pass1(nc, stage, wdnT_d, NKI, H, "wdn",
                                         consts)
            up_meanclip, up_swq = _prep_weight_scale(
                nc, stage, wps, consts, up_asum, H * I, "wup")
            dn_meanclip, dn_swq = _prep_weight_scale(
                nc, stage, wps, consts, dn_asum, H * I, "wdn")

            wup_q = []
            for k in range(NKH):
                wq = _prep_weight_pass2_tile(nc, stage, consts, wupT_d, k, I,
                                             "wup", magicb, up_swq, BF16)
                wup_q.append(wq.rearrange("p (a b) -> p a b", b=512))
            wdn_q = []
            for k in range(NKI):
                wq = _prep_weight_pass2_tile(nc, stage, consts, wdnT_d, k, H,
                                             "wdn", magicb, dn_swq, BF16)
                wdn_q.append(wq)

        # per-token gamma multiplier and final output multiplier
        k1b = consts.tile([P, 1], F32)
        nc.vector.tensor_scalar_mul(out=k1b, in0=up_meanclip,
                                    scalar1=1.0 / 127.0)
        wdk = consts.tile([P, 1], F32)
        nc.vector.tensor_scalar_mul(out=wdk, in0=dn_meanclip,
                                    scalar1=1.0 / 127.0)
        sg127 = consts.tile([P, 1], F32)
        nc.scalar.activation(out=sg127, in_=g0b, func=AF.Sign)
        nc.vector.tensor_scalar_mul(out=sg127, in0=sg127, scalar1=127.0)
        g0a = consts.tile([P, 1], F32)
        nc.scalar.activation(out=g0a, in_=g0b, func=AF.Abs)
        # isg = sign(g0)/127: d = 1/(Sabs*isg) = sign*127/Sabs
        isg = consts.tile([P, 1], F32)
        nc.vector.tensor_scalar_mul(out=isg, in0=sg127,
                                    scalar1=1.0 / (127.0 * 127.0))

        # -------- PSUM pools --------
        ps_xT = ctx.enter_context(tc.tile_pool(name="ps_xT", bufs=1,
                                               space="PSUM"))
        ps_ih = [ctx.enter_context(tc.tile_pool(name=f"ps_ih{c}", bufs=1,
                                                space="PSUM"))
                 for c in range(2)]
        ps_iuT = ctx.enter_context(tc.tile_pool(name="ps_iuT", bufs=1,
                                                space="PSUM"))
        ps_o = ctx.enter_context(tc.tile_pool(name="ps_o", bufs=1,
                                              space="PSUM"))
        IH2 = I // 2

        def up_a(x_sb, xsc8, jcol):
            """Quantize x, transpose, copy to SBUF (no ps_ih allocation)."""
            xqm = xq_pool.tile([P, H], F32, tag="xqm")
            nc.scalar.activation(out=xqm, in_=x_sb, func=AF.Identity,
                                 bias=magicb, scale=xsc8[:, jcol:jcol + 1])
            ix = hlp.tile([P, H], BF16, tag="ix")
            nc.scalar.activation(out=ix, in_=xqm, func=AF.Identity,
                                 bias=negmb)
            xT_ps = ps_xT.tile([P, NKH, P], BF16, tag="xT")
            for k in range(NKH):
                nc.tensor.transpose(out=xT_ps[:, k, :],
                                    in_=ix[:, k * P:(k + 1) * P],
                                    identity=ident)
            xT_sb = xTp.tile([P, NKH, P], BF16, tag="xTsb")
            nc.scalar.copy(out=xT_sb, in_=xT_ps)
            return xT_sb

        def up_b(xT_sb):
            """Up matmul into 2 psum half pools (allocated here, after the
            previous tile's relu drains were emitted). k-outer so each
            stationary xT k-tile is reused across the 4 output chunks."""
            halves = [ps_ih[h].tile([P, IH2], F32, tag=f"ih{h}",
                                    name=f"ih{h}") for h in range(2)]
            for k in range(NKH):
                for c in range(NB):
                    nc.tensor.matmul(out=halves[c // 2][:, (c % 2) * 512:
                                                        (c % 2) * 512 + 512],
                                     lhsT=xT_sb[:, k, :],
                                     rhs=wup_q[k][:, c, :],
                                     start=(k == 0), stop=(k == NKH - 1))
            return halves

        def down_a(halves, Sm8, jcol):
            """relu-drain the up psum halves to SBUF; per-half square (Pool)
            and max (DVE) start as soon as each half lands; then the
            per-token quant scale d."""
            r_sb = rp.tile([P, I], F32, tag="r")
            s_sb = rtp.tile([P, I], F32, tag="s")
            rm2 = smallp.tile([P, 2], F32, tag="rm2")
            for h in range(2):
                rh = r_sb[:, h * IH2:(h + 1) * IH2]
                nc.scalar.activation(out=rh, in_=halves[h], func=AF.Relu)
                nc.gpsimd.tensor_tensor(out=s_sb[:, h * IH2:(h + 1) * IH2],
                                        in0=rh, in1=rh, op=ALU.mult)
                nc.vector.tensor_reduce(out=rm2[:, h:h + 1], in_=rh,
                                        axis=mybir.AxisListType.X, op=ALU.max)
            mr = smallp.tile([P, 1], F32, tag="mr")
            nc.vector.tensor_reduce(out=mr, in_=rm2,
                                    axis=mybir.AxisListType.X, op=ALU.max)
            # Sabs = (max r)^2 (r >= 0)
            nc.vector.tensor_scalar(out=Sm8[:, jcol:jcol + 1], in0=mr,
                                    scalar1=mr, scalar2=None, op0=ALU.mult)
            # d = sign(g0)*127/clip(Sabs)
            sc2 = smallp.tile([P, 1], F32, tag="sc2")
            nc.vector.tensor_scalar(out=sc2, in0=Sm8[:, jcol:jcol + 1],
                                    scalar1=1e-30, scalar2=isg,
                                    op0=ALU.max, op1=ALU.mult)
            dr = smallp.tile([P, 1], F32, tag="dr")
            nc.vector.reciprocal(out=dr, in_=sc2)
            return s_sb, dr

        def down_b(s_sb, dr, q28, jcol, o_tiles):
            """Quantize, transpose, down mm, o drain."""
            rt = rtp.tile([P, I], F32, tag="rt")
            nc.vector.tensor_scalar(out=rt, in0=s_sb, scalar1=dr,
                                    scalar2=MAGIC, op0=ALU.mult, op1=ALU.add)
            iu = iup.tile([P, I], BF16, tag="iu")
            nc.vector.tensor_scalar(out=iu, in0=rt, scalar1=MAGIC,
                                    scalar2=None, op0=ALU.subtract)
            # q2 = sum(iu^2) via ACT square with accumulator
            junk = junkp.tile([P, I], BF16, tag="junk")
            nc.scalar.activation(out=junk, in_=iu, func=AF.Square,
                                 accum_out=q28[:, jcol:jcol + 1])
            # transpose iu
            iuT_ps = ps_iuT.tile([P, NKI, P], BF16, tag="iuT")
            for k in range(NKI):
                nc.tensor.transpose(out=iuT_ps[:, k, :],
                                    in_=iu[:, k * P:(k + 1) * P],
                                    identity=ident)
            iuT_sb = iuTp.tile([P, NKI, P], BF16, tag="iuTsb")
            nc.vector.tensor_copy(out=iuT_sb, in_=iuT_ps)
            # down matmul
            o_ps = ps_o.tile([P, H], F32, tag="o")
            for k in range(NKI):
                nc.tensor.matmul(out=o_ps, lhsT=iuT_sb[:, k, :],
                                 rhs=wdn_q[k],
                                 start=(k == 0), stop=(k == NKI - 1))
            o_sb = outp.tile([P, H], F32, tag="osb")
            nc.scalar.copy(out=o_sb, in_=o_ps)
            o_tiles.append(o_sb)

        def batch_beta_store(ib, Sm8, q28, t08, o_tiles):
            """Batched per-token output scale beta, then scale + store."""
            scc8 = batchp.tile([P, BG], F32, tag="scc8")
            nc.vector.tensor_scalar_max(out=scc8, in0=Sm8, scalar1=1e-30)
            ga8 = batchp.tile([P, BG], F32, tag="ga8")
            nc.vector.tensor_scalar_mul(out=ga8, in0=t08, scalar1=k1b)
            al8 = batchp.tile([P, BG], F32, tag="al8")
            nc.vector.tensor_tensor(out=al8, in0=ga8, in1=ga8, op=ALU.mult)
            m18 = batchp.tile([P, BG], F32, tag="m18")
            nc.vector.tensor_tensor(out=m18, in0=al8, in1=scc8, op=ALU.mult)
            m28 = batchp.tile([P, BG], F32, tag="m28")
            nc.vector.tensor_tensor(out=m28, in0=m18, in1=m18, op=ALU.mult)
            v18 = batchp.tile([P, BG], F32, tag="v18")
            nc.vector.tensor_tensor(out=v18, in0=m28, in1=q28, op=ALU.mult)
            Ve8 = batchp.tile([P, BG], F32, tag="Ve8")
            nc.vector.tensor_scalar(out=Ve8, in0=v18, scalar1=KV,
                                    scalar2=EPS, op0=ALU.mult, op1=ALU.add)
            sq8 = batchp.tile([P, BG], F32, tag="sq8")
            nc.scalar.activation(out=sq8, in_=Ve8, func=AF.Sqrt)
            cr8 = batchp.tile([P, BG], F32, tag="cr8")
            nc.vector.reciprocal(out=cr8, in_=sq8)
            h18 = batchp.tile([P, BG], F32, tag="h18")
            nc.vector.tensor_tensor(out=h18, in0=cr8, in1=cr8, op=ALU.mult)
            h28 = batchp.tile([P, BG], F32, tag="h28")
            nc.vector.tensor_tensor(out=h28, in0=h18, in1=Ve8, op=ALU.mult)
            h38 = batchp.tile([P, BG], F32, tag="h38")
            nc.vector.tensor_scalar(out=h38, in0=h28, scalar1=-0.5,
                                    scalar2=1.5, op0=ALU.mult, op1=ALU.add)
            c8 = batchp.tile([P, BG], F32, tag="c8")
            nc.vector.tensor_tensor(out=c8, in0=cr8, in1=h38, op=ALU.mult)
            m1g8 = batchp.tile([P, BG], F32, tag="m1g8")
            nc.vector.tensor_scalar_mul(out=m1g8, in0=m18, scalar1=g0a)
            mu8 = batchp.tile([P, BG], F32, tag="mu8")
            nc.vector.tensor_tensor(out=mu8, in0=c8, in1=m1g8, op=ALU.mult)
            b8 = batchp.tile([P, BG], F32, tag="b8")
            nc.vector.tensor_scalar(out=b8, in0=mu8, scalar1=1e-5,
                                    scalar2=wdk, op0=ALU.max, op1=ALU.mult)
            for j in range(BG):
                r0 = (ib + j) * P
                o2 = o2p.tile([P, H], F32, tag="o2")
                nc.vector.tensor_scalar_mul(out=o2, in0=o_tiles[j],
                                            scalar1=b8[:, j:j + 1])
                nc.sync.dma_start(out=out_d[r0:r0 + P, :], in_=o2)

        # -------- software-pipelined main loop (pipeline carries across
        # batch boundaries so the PE never drains between batches) --------
        pend = None   # (ih_halves, Sm8, q28, o_tiles, jcol, ib, t08)
        cur = None
        for jj in range(NT):
            ib = (jj // BG) * BG
            j = jj % BG
            if j == 0:
                x_tiles, t08, xsc8 = state.pop(ib)
                if ib + 2 * BG < NT:
                    phase_a(ib + 2 * BG)
                Sm8 = batchp.tile([P, BG], F32, tag="Sm8")
                q28 = batchp.tile([P, BG], F32, tag="q28")
                cur = dict(Sm8=Sm8, q28=q28, o_tiles=[], ib=ib, t08=t08)
            if pend is not None:
                p_ih, p_cur, p_j = pend
                s_p, dr_p = down_a(p_ih, p_cur["Sm8"], p_j)
            xT_sb = up_a(x_tiles[j], xsc8, j)
            ih = up_b(xT_sb)
            if pend is not None:
                down_b(s_p, dr_p, p_cur["q28"], p_j, p_cur["o_tiles"])
                if p_j == BG - 1:
                    batch_beta_store(p_cur["ib"], p_cur["Sm8"],
                                     p_cur["q28"], p_cur["t08"],
                                     p_cur["o_tiles"])
            pend = (ih, cur, j)
        # drain the last tile
        p_ih, p_cur, p_j = pend
        s_p, dr_p = down_a(p_ih, p_cur["Sm8"], p_j)
        down_b(s_p, dr_p, p_cur["q28"], p_j, p_cur["o_tiles"])
        batch_beta_store(p_cur["ib"], p_cur["Sm8"], p_cur["q28"],
                         p_cur["t08"], p_cur["o_tiles"])

    _split_sync_waits(nc)
    return nc


# ===========================================================================
# Fallback build for non-constant g (not exercised by the harness inputs,
# kept for correctness): original implementation.
# ===========================================================================

def _emit_weight_quant_g(nc, stage, junkp, ps, consts, wT_dram, n_ktiles,
                         nsub, name, magicb):
    n_elem = n_ktiles * 128 * nsub * 512
    asum = consts.tile([P, n_ktiles], F32, tag=f"{name}_asum")
    for k in range(n_ktiles):
        wf = stage.tile([P, nsub * 512], F32, tag="stage")
        nc.gpsimd.dma_start(out=wf, in_=wT_dram[k * P:(k + 1) * P, :])
        junk = junkp.tile([P, nsub * 512], BF16, tag="junk")
        nc.scalar.activation(out=junk, in_=wf, func=AF.Abs,
                             accum_out=asum[:, k:k + 1])
    tot = consts.tile([P, 1], F32, tag=f"{name}_tot")
    nc.vector.tensor_reduce(out=tot, in_=asum, axis=mybir.AxisListType.X,
                            op=ALU.add)
    ones128 = stage.tile([P, P], F32, tag="ones128")
    nc.vector.memset(ones128, 1.0)
    totp = ps.tile([P, 1], F32, tag="totp")
    nc.tensor.matmul(out=totp, lhsT=ones128, rhs=tot, start=True, stop=True)
    gsum = consts.tile([P, 1], F32, tag=f"{name}_gsum")
    nc.scalar.copy(out=gsum, in_=totp)
    meanclip = consts.tile([P, 1], F32, tag=f"{name}_meanclip")
    nc.vector.tensor_scalar(out=meanclip, in0=gsum, scalar1=1.0 / n_elem,
                            scalar2=1e-5, op0=ALU.mult, op1=ALU.max)
    swq = consts.tile([P, 1], F32, tag=f"{name}_swq")
    nc.vector.reciprocal(out=swq, in_=meanclip)

    wq_tiles = []
    for k in range(n_ktiles):
        wf = stage.tile([P, nsub * 512], F32, tag="stage")
        nc.gpsimd.dma_start(out=wf, in_=wT_dram[k * P:(k + 1) * P, :])
        rt = stage.tile([P, nsub * 512], F32, tag="stage_rt")
        nc.scalar.activation(out=rt, in_=wf, func=AF.Identity,
                             bias=magicb, scale=swq)
        cl = stage.tile([P, nsub * 512], F32, tag="stage_cl")
        nc.vector.tensor_scalar(out=cl, in0=rt, scalar1=MAGIC, scalar2=1.0,
                                op0=ALU.subtract, op1=ALU.min)
        wq = consts.tile([P, nsub, 512], BF16, tag=f"{name}_wq{k}")
        nc.vector.tensor_scalar(out=wq.rearrange("p a b -> p (a b)"), in0=cl,
                                scalar1=-1.0, scalar2=None, op0=ALU.max)
        wq_tiles.append(wq)
    return wq_tiles, meanclip


def build_nc_general():
    nc = bass.Bass()
    x_d = nc.dram_tensor("x", [TPC, H], F32, kind="ExternalInput")
    wupT_d = nc.dram_tensor("wupT", [H, I], F32, kind="ExternalInput")
    wdnT_d = nc.dram_tensor("wdnT", [I, H], F32, kind="ExternalInput")
    g_d = nc.dram_tensor("g", [I], F32, kind="ExternalInput")
    out_d = nc.dram_tensor("out", [TPC, H], F32, kind="ExternalOutput")

    from contextlib import ExitStack
    with ExitStack() as ctx:
        tc = ctx.enter_context(tile.TileContext(nc))
        consts = ctx.enter_context(tc.tile_pool(name="consts", bufs=1))

        ident = consts.tile([P, P], BF16)
        make_identity(nc, ident)
        magicb = consts.tile([P, 1], F32)
        nc.vector.memset(magicb, MAGIC)

        g_bc = consts.tile([P, I], F32)
        g_ap = g_d[:]
        g_bcast_ap = bass.AP(tensor=g_ap.tensor, offset=g_ap.offset,
                             ap=[[0, P]] + list(g_ap.ap))
        nc.gpsimd.dma_start(out=g_bc, in_=g_bcast_ap)

        with tc.tile_pool(name="wstage", bufs=2) as stage, \
                tc.tile_pool(name="wjunk", bufs=2) as junkp, \
                tc.tile_pool(name="wps", bufs=1, space="PSUM") as wps:
            wup_q, up_meanclip = _emit_weight_quant_g(
                nc, stage, junkp, wps, consts, wupT_d, NKH, NB, "wup", magicb)
            wdn_q, dn_meanclip = _emit_weight_quant_g(
                nc, stage, junkp, wps, consts, wdnT_d, NKI, 1, "wdn", magicb)

        k1b = consts.tile([P, 1], F32)
        nc.vector.tensor_scalar_mul(out=k1b, in0=up_meanclip,
                                    scalar1=1.0 / 127.0)
        wdk = consts.tile([P, 1], F32)
        nc.vector.tensor_scalar_mul(out=wdk, in0=dn_meanclip,
                                    scalar1=1.0 / 127.0)
        isg = consts.tile([P, 1], F32)
        nc.vector.memset(isg, 1.0 / 127.0)

        BG = 8
        xs_pool = ctx.enter_context(tc.tile_pool(name="xs", bufs=2 * BG))
        xq_pool = ctx.enter_context(tc.tile_pool(name="xqp", bufs=3))
        big = ctx.enter_context(tc.tile_pool(name="big", bufs=2))
        iup = ctx.enter_context(tc.tile_pool(name="iup", bufs=3))
        outp = ctx.enter_context(tc.tile_pool(name="outp", bufs=BG + 1))
        o2p = ctx.enter_context(tc.tile_pool(name="o2p", bufs=3))
        junkp = ctx.enter_context(tc.tile_pool(name="mjunk", bufs=1))
        small = ctx.enter_context(tc.tile_pool(name="small", bufs=3))
        batchp = ctx.enter_context(tc.tile_pool(name="batchp", bufs=2))
        ps_xT = ctx.enter_context(tc.tile_pool(name="ps_xT", bufs=1,
                                               space="PSUM"))
        ps_ih = ctx.enter_context(tc.tile_pool(name="ps_ih", bufs=1,
                                               space="PSUM"))
        ps_iuT = ctx.enter_context(tc.tile_pool(name="ps_iuT", bufs=1,
                                                space="PSUM"))
        ps_o = ctx.enter_context(tc.tile_pool(name="ps_o", bufs=1,
                                              space="PSUM"))

        IH2 = I // 2
        state = {}

        def phase_a(ib):
            xm8 = batchp.tile([P, BG], F32, tag="xm8")
            x_tiles = []
            for j in range(BG):
                r0 = (ib + j) * P
                x_sb = xs_pool.tile([P, H], F32, tag="x")
                nc.sync.dma_start(out=x_sb, in_=x_d[r0:r0 + P, :])
                x_tiles.append(x_sb)
                nc.vector.tensor_reduce(out=xm8[:, j:j + 1], in_=x_sb,
                                        axis=mybir.AxisListType.X, op=ALU.max,
                                        apply_absolute_value=True)
            t08 = batchp.tile([P, BG], F32, tag="t08")
            nc.vector.tensor_scalar_max(out=t08, in0=xm8, scalar1=1e-5)
            xr8 = batchp.tile([P, BG], F32, tag="xr8")
            nc.vector.reciprocal(out=xr8, in_=t08)
            xsc8 = batchp.tile([P, BG], F32, tag="xsc8")
            nc.vector.tensor_scalar_mul(out=xsc8, in0=xr8, scalar1=127.0)
            state[ib] = (x_tiles, t08, xsc8)

        def phase_bc(ib):
            x_tiles, t08, xsc8 = state.pop(ib)
            Sm8 = batchp.tile([P, BG], F32, tag="Sm8")
            q2g8 = batchp.tile([P, BG], F32, tag="q2g8")
            o_tiles = []

            for j in range(BG):
                x_sb = x_tiles[j]
                xq = xq_pool.tile([P, H], F32, tag="xq")
                nc.scalar.activation(out=xq, in_=x_sb, func=AF.Identity,
                                     bias=magicb, scale=xsc8[:, j:j + 1])
                ix = xq_pool.tile([P, H], BF16, tag="ix")
                nc.vector.tensor_scalar(out=ix, in0=xq, scalar1=MAGIC,
                                        scalar2=None, op0=ALU.subtract)
                xT_ps = ps_xT.tile([P, NKH, P], BF16, tag="xT")
                for k in range(NKH):
                    nc.tensor.transpose(out=xT_ps[:, k, :],
                                        in_=ix[:, k * P:(k + 1) * P],
                                        identity=ident)
                xT_sb = xq_pool.tile([P, NKH, P], BF16, tag="xTsb")
                nc.scalar.copy(out=xT_sb, in_=xT_ps)

                r_sb = big.tile([P, I], F32, tag="r")
                for h in range(2):
                    ihh = ps_ih.tile([P, IH2], F32, tag="ih")
                    for nb in range(2):
                        lo = nb * 512
                        for k in range(NKH):
                            nc.tensor.matmul(
                                out=ihh[:, lo:lo + 512],
                                lhsT=xT_sb[:, k, :],
                                rhs=wup_q[k][:, 2 * h + nb, :],
                                start=(k == 0), stop=(k == NKH - 1))
                    nc.scalar.activation(out=r_sb[:, h * IH2:(h + 1) * IH2],
                                         in_=ihh, func=AF.Relu)

                s_sb = big.tile([P, I], F32, tag="s")
                nc.gpsimd.tensor_tensor(out=s_sb, in0=r_sb, in1=r_sb,
                                        op=ALU.mult)
                sq_in = big.tile([P, I], F32, tag="sg")
                nc.vector.tensor_tensor(out=sq_in, in0=s_sb, in1=g_bc,
                                        op=ALU.mult)
                junk3 = junkp.tile([P, I], BF16, tag="junk3")
                nc.scalar.activation(out=junk3, in_=s_sb, func=AF.Square,
                                     accum_out=q2g8[:, j:j + 1])
                nc.vector.tensor_reduce(out=Sm8[:, j:j + 1], in_=sq_in,
                                        axis=mybir.AxisListType.X,
                                        op=ALU.max,
                                        apply_absolute_value=True)
                sc2 = small.tile([P, 1], F32, tag="sc2")
                nc.vector.tensor_scalar(out=sc2, in0=Sm8[:, j:j + 1],
                                        scalar1=1e-30, scalar2=isg,
                                        op0=ALU.max, op1=ALU.mult)
                dr = small.tile([P, 1], F32, tag="dr")
                nc.vector.reciprocal(out=dr, in_=sc2)
                rt = big.tile([P, I], F32, tag="rt")
                nc.vector.tensor_scalar(out=rt, in0=sq_in, scalar1=dr,
                                        scalar2=MAGIC, op0=ALU.mult,
                                        op1=ALU.add)
                iu = iup.tile([P, I], BF16, tag="iu")
                nc.vector.tensor_scalar(out=iu, in0=rt, scalar1=MAGIC,
                                        scalar2=None, op0=ALU.subtract)

                iuT_ps = ps_iuT.tile([P, NKI, P], BF16, tag="iuT")
                for k in range(NKI):
                    nc.tensor.transpose(out=iuT_ps[:, k, :],
                                        in_=iu[:, k * P:(k + 1) * P],
                                        identity=ident)
                iuT_sb = iup.tile([P, NKI, P], BF16, tag="iuTsb")
                nc.scalar.copy(out=iuT_sb, in_=iuT_ps)

                o_ps = ps_o.tile([P, H], F32, tag="o")
                for k in range(NKI):
                    nc.tensor.matmul(out=o_ps, lhsT=iuT_sb[:, k, :],
                                     rhs=wdn_q[k][:, 0, :],
                                     start=(k == 0), stop=(k == NKI - 1))
                o_sb = outp.tile([P, H], F32, tag="osb")
                nc.scalar.copy(out=o_sb, in_=o_ps)
                o_tiles.append(o_sb)

            scc8 = batchp.tile([P, BG], F32, tag="scc8")
            nc.vector.tensor_scalar_max(out=scc8, in0=Sm8, scalar1=1e-30)
            ga8 = batchp.tile([P, BG], F32, tag="ga8")
            nc.vector.tensor_scalar_mul(out=ga8, in0=t08, scalar1=k1b)
            al8 = batchp.tile([P, BG], F32, tag="al8")
            nc.vector.tensor_tensor(out=al8, in0=ga8, in1=ga8, op=ALU.mult)
            m18 = batchp.tile([P, BG], F32, tag="m18")
            nc.vector.tensor_tensor(out=m18, in0=al8, in1=scc8, op=ALU.mult)
            v18 = batchp.tile([P, BG], F32, tag="v18")
            Ve8 = batchp.tile([P, BG], F32, tag="Ve8")
            al28 = batchp.tile([P, BG], F32, tag="al28")
            nc.vector.tensor_tensor(out=al28, in0=al8, in1=al8, op=ALU.mult)
            nc.vector.tensor_tensor(out=v18, in0=al28, in1=q2g8, op=ALU.mult)
            nc.vector.tensor_scalar(out=Ve8, in0=v18, scalar1=1.0 / I,
                                    scalar2=EPS, op0=ALU.mult, op1=ALU.add)
            sq8 = batchp.tile([P, BG], F32, tag="sq8")
            nc.scalar.activation(out=sq8, in_=Ve8, func=AF.Sqrt)
            cr8 = batchp.tile([P, BG], F32, tag="cr8")
            nc.vector.reciprocal(out=cr8, in_=sq8)
            h18 = batchp.tile([P, BG], F32, tag="h18")
            nc.vector.tensor_tensor(out=h18, in0=cr8, in1=cr8, op=ALU.mult)
            h28 = batchp.tile([P, BG], F32, tag="h28")
            nc.vector.tensor_tensor(out=h28, in0=h18, in1=Ve8, op=ALU.mult)
            h38 = batchp.tile([P, BG], F32, tag="h38")
            nc.vector.tensor_scalar(out=h38, in0=h28, scalar1=-0.5,
                                    scalar2=1.5, op0=ALU.mult, op1=ALU.add)
            c8 = batchp.tile([P, BG], F32, tag="c8")
            nc.vector.tensor_tensor(out=c8, in0=cr8, in1=h38, op=ALU.mult)
            mu8 = batchp.tile([P, BG], F32, tag="mu8")
            nc.vector.tensor_tensor(out=mu8, in0=c8, in1=m18, op=ALU.mult)
            b8 = batchp.tile([P, BG], F32, tag="b8")
            nc.vector.tensor_scalar(out=b8, in0=mu8, scalar1=1e-5,
                                    scalar2=wdk, op0=ALU.max, op1=ALU.mult)

            for j in range(BG):
                r0 = (ib + j) * P
                o2 = o2p.tile([P, H], F32, tag="o2")
                nc.vector.tensor_scalar_mul(out=o2, in0=o_tiles[j],
                                            scalar1=b8[:, j:j + 1])
                nc.sync.dma_start(out=out_d[r0:r0 + P, :], in_=o2)

        phase_a(0)
        for ib in range(0, NT, BG):
            if ib + BG < NT:
                phase_a(ib + BG)
            phase_bc(ib)

    _split_sync_waits(nc)
    return nc


_NC_CACHE = {}


def kernel(x, w_up, w_down, g):
    global LAST_RESULT
    x = np.ascontiguousarray(x, dtype=np.float32)
    w_up = np.ascontiguousarray(w_up, dtype=np.float32)
    w_down = np.ascontiguousarray(w_down, dtype=np.float32)
    g = np.ascontiguousarray(g, dtype=np.float32)

    if abs(float(g[0])) < 1e-30 and np.all(g == g[0]):
        return np.zeros_like(x)

    general = not bool(np.all(g == g[0]))
    key = ("gen" if general else "const")
    if key not in _NC_CACHE:
        _NC_CACHE[key] = (build_nc_general() if general
                          else build_nc_const())
    nc = _NC_CACHE[key]

    xt = x.reshape(TOK, H)
    wupT = np.ascontiguousarray(w_up.T)    # [H, I]
    wdnT = np.ascontiguousarray(w_down.T)  # [I, H]
    in_maps = [
        {"x": xt[c * TPC:(c + 1) * TPC], "wupT": wupT, "wdnT": wdnT, "g": g}
        for c in range(N_CORES)
    ]
    res = run_bass_kernel_spmd(
        nc, in_maps, list(range(N_CORES)),
        trace=bool(os.environ.get("BASS_TRACE")),
    )
    LAST_RESULT = res
    out = np.concatenate([res.results[c]["out"] for c in range(N_CORES)],
                         axis=0)
    return out.reshape(B, S, H)


# revision 33
# speedup vs baseline: 1.0278x; 1.0278x over previous
"""BitNet MLP (act_quant -> ternary matmul -> relu^2 -> SubLN -> act_quant ->
ternary matmul) on 8 Trainium2 NeuronCores, data-parallel over tokens.

Math notes (exactness):
- act_quant int levels (|q| <= 127) and ternary weights {-1,0,1} are exactly
  representable in bf16, so both matmuls run on the PE in bf16 with exact
  integer arithmetic (f32 PSUM accumulation, |sums| < 2^24).
- All quantization scales are folded into per-token scalars applied to the
  final [tok, 512] output: out = i2 * beta_t with
    beta_t = clip(c_t * alpha_t * Sabs_t, 1e-5) * clip(mean|w_dn|,1e-5) / 127
  where alpha_t = (clip(max|x_t|,1e-5) * clip(mean|w_up|,1e-5) / 127)^2,
  Sabs_t = max_i |relu(ih)^2 * g|, c_t = rsqrt(var_t + 1e-6).
- Rounding uses the magic-number trick (x + 1.5*2^23 - 1.5*2^23) == RNE
  round-to-integer for |x| < 2^22, matching jnp.round (half-to-even).
- SubLN variance is recovered from the quantized intermediate:
  var = alpha^2 * sum(iu^2) * (Sabs/127)^2 / (2048 * g0^2); the
  quantization error on sum(iu^2) is ~0.1% which is far below tolerance.
  (For non-constant g an extra pass computes sum((relu^2)^2) directly.)
"""
import os
import numpy as np

import concourse.bass as bass
import concourse.tile as tile
from concourse import mybir
from concourse.bass_utils import run_bass_kernel_spmd
from concourse.masks import make_identity

# ---------------------------------------------------------------------------
# Workaround for walrus "Too many sync wait commands" on the TileContext tail
# drain: split the drain's semaphore waits across single-wait SP NOPs, then
# advance the observed clocks so the real drain needs none.
import re as _re
import bass_rust as _bass_rust


def _patched_drain_and_barrier(self, tick_clock, wait_clock):
    gc = tick_clock.global_clock
    ticks = list(map(int, _re.findall(r"\d+", repr(gc))))
    n = len(ticks)
    nonzero = [(i, t) for i, t in enumerate(ticks) if t > 0]
    for i, t in nonzero:
        sub = [0] * n
        sub[i] = t
        sub_scoped = _bass_rust.ScopedClock({None: _bass_rust.VectorClock(sub)})
        nop = self.nc.sync.nop()
        wait_clock.add_sem_waits(nop.ins, sub_scoped)
        for ec in wait_clock.engine_clocks:
            ec.update_past(sub_scoped)
    drain_inst = self.nc.sync.drain()
    wait_clock.add_sem_waits(drain_inst.ins,
                             _bass_rust.ScopedClock({None: gc}))
    self.nc.all_engine_barrier()
    popped = self.nc._tile_sem_poison_stack.pop()
    assert popped is self._sem_poison
    self.nc.clear_and_free_semaphores(list(self.sems.allocated().values()))
    self.nc.all_engine_barrier()


tile.TileContext._drain_and_barrier = _patched_drain_and_barrier


def _split_sync_waits(nc, keep_default=1):
    """walrus caps the number of semaphore waits a single instruction can
    carry (CTRL ops take only 1; compute ops a few). Hoist excess waits onto
    single-wait NOPs inserted immediately before the instruction on the same
    engine — identical semantics, engines execute in order."""
    import dataclasses
    keep_by_op = {}
    proto = None
    for f in nc.m.functions:
        for bb in f.blocks:
            for inst in bb.instructions:
                if type(inst).__name__ == "InstNoOp":
                    proto = inst
                    break
            if proto is not None:
                break
        if proto is not None:
            break
    counter = [0]
    for f in nc.m.functions:
        new_blocks = []
        for bb in f.blocks:
            out = []
            changed = False
            for inst in bb.instructions:
                si = inst.sync_info
                ow = list(si.on_wait) if si is not None and si.on_wait else []
                keep = keep_by_op.get(inst.opcode, keep_default)
                if len(ow) > keep:
                    assert proto is not None, "no NoOp prototype found yet"
                    for w in ow[:-keep]:
                        counter[0] += 1
                        nop = dataclasses.replace(
                            proto,
                            name=f"I-waitsplit-{counter[0]}",
                            engine=inst.engine,
                            sync_info=_bass_rust.SyncInfo(on_wait=[w],
                                                          on_update=[]),
                        )
                        out.append(nop)
                    si.on_wait = ow[-keep:]
                    changed = True
                out.append(inst)
            if changed:
                bb2 = _bass_rust.BasicBlock(name=bb.name, instructions=out)
                bb2.IsExit = bb.IsExit
                bb2.IsLoopEntry = bb.IsLoopEntry
                bb2.IsPredicated = bb.IsPredicated
                new_blocks.append(bb2)
            else:
                new_blocks.append(bb)
        f.blocks = new_blocks
# ---------------------------------------------------------------------------

F32 = mybir.dt.float32
BF16 = mybir.dt.bfloat16
ALU = mybir.AluOpType
AF = mybir.ActivationFunctionType

N_CORES = 8
B, S, H, I = 8, 8192, 512, 2048
TOK = B * S                  # 65536 tokens total
TPC = TOK // N_CORES         # 8192 tokens per core
P = 128                      # partition tile
NT = TPC // P                # 64 token tiles per core
NKH = H // P                 # 4 k-tiles over H
NKI = I // P                 # 16 k-tiles over I
NB = I // 512                # 4 psum banks for the up matmul

MAGIC = 12582912.0           # 1.5 * 2^23: RNE round-to-int trick
EPS = 1e-6                   # SubLN eps (from reference)

LAST_RESULT = None           # set by kernel() for test harness introspection


# ===========================================================================
# Optimized build (constant g, the case the harness exercises)
# ===========================================================================

_DMA_QUEUES = None


def _dma_queue(nc, i):
    """Round-robin weight-prep DMAs across engine queues so the transfers
    overlap instead of serializing on one ring."""
    return [nc.gpsimd, nc.scalar, nc.sync][i % 3]


def _prep_weight_pass1(nc, stage, wT_dram, n_ktiles, width, name, consts):
    """DMA the weight tiles and accumulate per-partition |w| sums (DVE)."""
    asum = consts.tile([P, n_ktiles], F32, tag=f"{name}_asum")
    for k in range(n_ktiles):
        wf = stage.tile([P, width], F32, tag="wst")
        _dma_queue(nc, k).dma_start(out=wf, in_=wT_dram[k * P:(k + 1) * P, :])
        nc.vector.tensor_reduce(out=asum[:, k:k + 1], in_=wf,
                                axis=mybir.AxisListType.X, op=ALU.add,
                                apply_absolute_value=True)
    return asum


def _prep_weight_scale(nc, stage, ps, consts, asum, n_elem, name):
    """Reduce per-partition sums to the global clip(mean|w|,1e-5) broadcast
    to all partitions, plus its reciprocal."""
    tot = consts.tile([P, 1], F32, tag=f"{name}_tot")
    nc.vector.tensor_reduce(out=tot, in_=asum, axis=mybir.AxisListType.X,
                            op=ALU.add)
    ones128 = stage.tile([P, P], F32, tag=f"{name}_ones")
    nc.vector.memset(ones128, 1.0)
    totp = ps.tile([P, 1], F32, tag=f"{name}_totp")
    nc.tensor.matmul(out=totp, lhsT=ones128, rhs=tot, start=True, stop=True)
    gsum = consts.tile([P, 1], F32, tag=f"{name}_gsum")
    nc.scalar.copy(out=gsum, in_=totp)
    meanclip = consts.tile([P, 1], F32, tag=f"{name}_meanclip")
    nc.vector.tensor_scalar(out=meanclip, in0=gsum, scalar1=1.0 / n_elem,
                            scalar2=1e-5, op0=ALU.mult, op1=ALU.max)
    swq = consts.tile([P, 1], F32, tag=f"{name}_swq")
    nc.vector.reciprocal(out=swq, in_=meanclip)
    return meanclip, swq


def _prep_weight_pass2_tile(nc, stage, consts, wT_dram, k, width, name,
                            magicb, swq, out_dt):
    """Re-DMA one k-tile and quantize to ternary in dtype out_dt.
    Returns a [P, width] tile."""
    wf = stage.tile([P, width], F32, tag="wst")
    _dma_queue(nc, k).dma_start(out=wf, in_=wT_dram[k * P:(k + 1) * P, :])
    rt = stage.tile([P, width], F32, tag="wrt")
    nc.scalar.activation(out=rt, in_=wf, func=AF.Identity,
                         bias=magicb, scale=swq)
    nc.vector.tensor_scalar(out=rt, in0=rt, scalar1=MAGIC, scalar2=1.0,
                            op0=ALU.subtract, op1=ALU.min)
    wq = consts.tile([P, width], out_dt, tag=f"{name}_wq{k}")
    nc.vector.tensor_scalar(out=wq, in0=rt, scalar1=-1.0, scalar2=None,
                            op0=ALU.max)
    return wq


def build_nc_const():
    nc = bass.Bass()
    x_d = nc.dram_tensor("x", [TPC, H], F32, kind="ExternalInput")
    wupT_d = nc.dram_tensor("wupT", [H, I], F32, kind="ExternalInput")
    wdnT_d = nc.dram_tensor("wdnT", [I, H], F32, kind="ExternalInput")
    g_d = nc.dram_tensor("g", [I], F32, kind="ExternalInput")
    out_d = nc.dram_tensor("out", [TPC, H], F32, kind="ExternalOutput")

    BG = 8                    # tiles per small-op batch
    KV = 1.0 / (127.0 * 127.0 * I)

    from contextlib import ExitStack
    with ExitStack() as ctx:
        tc = ctx.enter_context(tile.TileContext(nc))

        consts = ctx.enter_context(tc.tile_pool(name="consts", bufs=1))

        ident = consts.tile([P, P], BF16)
        make_identity(nc, ident)
        magicb = consts.tile([P, 1], F32)
        nc.vector.memset(magicb, MAGIC)
        negmb = consts.tile([P, 1], F32)
        nc.vector.memset(negmb, -MAGIC)

        # -------- main-loop pools (created before prep so x prefetch can
        # land while weights stream in) --------
        xs_pool = ctx.enter_context(tc.tile_pool(name="xs", bufs=2 * BG))
        xq_pool = ctx.enter_context(tc.tile_pool(name="xqp", bufs=2))
        hlp = ctx.enter_context(tc.tile_pool(name="hlp", bufs=2))
        xTp = ctx.enter_context(tc.tile_pool(name="xTp", bufs=2))
        rp = ctx.enter_context(tc.tile_pool(name="rp", bufs=2))
        rtp = ctx.enter_context(tc.tile_pool(name="rtp", bufs=2))
        iup = ctx.enter_context(tc.tile_pool(name="iup", bufs=2))
        iuTp = ctx.enter_context(tc.tile_pool(name="iuTp", bufs=2))
        junkp = ctx.enter_context(tc.tile_pool(name="junkp", bufs=1))
        smallp = ctx.enter_context(tc.tile_pool(name="smallp", bufs=4))
        outp = ctx.enter_context(tc.tile_pool(name="outp", bufs=BG + 1))
        o2p = ctx.enter_context(tc.tile_pool(name="o2p", bufs=3))
        batchp = ctx.enter_context(tc.tile_pool(name="batchp", bufs=2))

        # -------- x prefetch state --------
        state = {}

        def phase_a(ib):
            """DMA x tiles + per-token absmax, then batched x-scale chain."""
            xm8 = batchp.tile([P, BG], F32, tag="xm8")
            x_tiles = []
            for j in range(BG):
                r0 = (ib + j) * P
                x_sb = xs_pool.tile([P, H], F32, tag="x")
                nc.sync.dma_start(out=x_sb, in_=x_d[r0:r0 + P, :])
                x_tiles.append(x_sb)
                nc.vector.tensor_reduce(out=xm8[:, j:j + 1], in_=x_sb,
                                        axis=mybir.AxisListType.X, op=ALU.max,
                                        apply_absolute_value=True)
            t08 = batchp.tile([P, BG], F32, tag="t08")
            nc.vector.tensor_scalar_max(out=t08, in0=xm8, scalar1=1e-5)
            xr8 = batchp.tile([P, BG], F32, tag="xr8")
            nc.vector.reciprocal(out=xr8, in_=t08)
            xsc8 = batchp.tile([P, BG], F32, tag="xsc8")
            nc.vector.tensor_scalar_mul(out=xsc8, in0=xr8, scalar1=127.0)
            state[ib] = (x_tiles, t08, xsc8)

        # issue the first two batches of x loads before weight prep so the
        # DMA and absmax work overlaps the weight streaming
        phase_a(0)
        phase_a(BG)

        # -------- weight prep --------
        with tc.tile_pool(name="wstage", bufs=2) as stage, \
                tc.tile_pool(name="wps", bufs=1, space="PSUM") as wps:
            # g0 broadcast [128,1] via K=1 matmul with ones (g is constant)
            ones_row = stage.tile([1, P], F32, tag="ones_row")
            nc.vector.memset(ones_row, 1.0)
            g0_sb = stage.tile([1, 1], F32, tag="g0sb")
            nc.gpsimd.dma_start(out=g0_sb, in_=g_d[0:1])
            g0_ps = wps.tile([P, 1], F32, tag="g0ps")
            nc.tensor.matmul(out=g0_ps, lhsT=ones_row, rhs=g0_sb, start=True,
                             stop=True)
            g0b = consts.tile([P, 1], F32)
            nc.scalar.copy(out=g0b, in_=g0_ps)

            up_asum = _prep_weight_pass1(nc, stage, wupT_d, NKH, I, "wup",
                                         consts)
            dn_asum = _prep_weight_pass1(nc, stage, wdnT_d, NKI, H, "wdn",
                                         consts)
            up_meanclip, up_swq = _prep_weight_scale(
                nc, stage, wps, consts, up_asum, H * I, "wup")
            dn_meanclip, dn_swq = _prep_weight_scale(
                nc, stage, wps, consts, dn_asum, H * I, "wdn")

            wup_q = []
            for k in range(NKH):
                wq = _prep_weight_pass2_tile(nc, stage, consts, wupT_d, k, I,
                                             "wup", magicb, up_swq, BF16)
                wup_q.append(wq.rearrange("p (a b) -> p a b", b=512))
            wdn_q = []
            for k in range(NKI):
                wq = _prep_weight_pass2_tile(nc, stage, consts, wdnT_d, k, H,
                                             "wdn", magicb, dn_swq, BF16)
                wdn_q.append(wq)

        # per-token gamma multiplier and final output multiplier
        k1b = consts.tile([P, 1], F32)
        nc.vector.tensor_scalar_mul(out=k1b, in0=up_meanclip,
                                    scalar1=1.0 / 127.0)
        wdk = consts.tile([P, 1], F32)
        nc.vector.tensor_scalar_mul(out=wdk, in0=dn_meanclip,
                                    scalar1=1.0 / 127.0)
        sg127 = consts.tile([P, 1], F32)
        nc.scalar.activation(out=sg127, in_=g0b, func=AF.Sign)
        nc.vector.tensor_scalar_mul(out=sg127, in0=sg127, scalar1=127.0)
        g0a = consts.tile([P, 1], F32)
        nc.scalar.activation(out=g0a, in_=g0b, func=AF.Abs)
        # isg = sign(g0)/127: d = 1/(Sabs*isg) = sign*127/Sabs
        isg = consts.tile([P, 1], F32)
        nc.vector.tensor_scalar_mul(out=isg, in0=sg127,
                                    scalar1=1.0 / (127.0 * 127.0))

        # -------- PSUM pools --------
        ps_xT = ctx.enter_context(tc.tile_pool(name="ps_xT", bufs=1,
                                               space="PSUM"))
        ps_ih = [ctx.enter_context(tc.tile_pool(name=f"ps_ih{c}", bufs=1,
                                                space="PSUM"))
                 for c in range(2)]
        ps_iuT = ctx.enter_context(tc.tile_pool(name="ps_iuT", bufs=1,
                                                space="PSUM"))
        ps_o = ctx.enter_context(tc.tile_pool(name="ps_o", bufs=1,
                                              space="PSUM"))
        IH2 = I // 2

        def up_a(x_sb, xsc8, jcol):
            """Quantize x, transpose, copy to SBUF (no ps_ih allocation)."""
            xqm = xq_pool.tile([P, H], F32, tag="xqm")
            nc.scalar.activation(out=xqm, in_=x_sb, func=AF.Identity,
                                 bias=magicb, scale=xsc8[:, jcol:jcol + 1])
            ix = hlp.tile([P, H], BF16, tag="ix")
            nc.scalar.activation(out=ix, in_=xqm, func=AF.Identity,
                                 bias=negmb)
            xT_ps = ps_xT.tile([P, NKH, P], BF16, tag="xT")
            for k in range(NKH):
                nc.tensor.transpose(out=xT_ps[:, k, :],
                                    in_=ix[:, k * P:(k + 1) * P],
                                    identity=ident)
            xT_sb = xTp.tile([P, NKH, P], BF16, tag="xTsb")
            nc.scalar.copy(out=xT_sb, in_=xT_ps)
            return xT_sb

        def up_b(xT_sb):
            """Up matmul into 2 psum half pools (allocated here, after the
            previous tile's relu drains were emitted). k-outer so each
            stationary xT k-tile is reused across the 4 output chunks."""
            halves = [ps_ih[h].tile([P, IH2], F32, tag=f"ih{h}",
                                    name=f"ih{h}") for h in range(2)]
            for k in range(NKH):
                for c in range(NB):
                    nc.tensor.matmul(out=halves[c // 2][:, (c % 2) * 512:
                                                        (c % 2) * 512 + 512],
                                     lhsT=xT_sb[:, k, :],
                                     rhs=wup_q[k][:, c, :],
                                     start=(k == 0), stop=(k == NKH - 1))
            return halves

        def down_a(halves, Sm8, jcol):
            """relu-drain the up psum halves to SBUF; per-half square (Pool)
            and max (DVE) start as soon as each half lands; then the
            per-token quant scale d."""
            r_sb = rp.tile([P, I], F32, tag="r")
            s_sb = rtp.tile([P, I], F32, tag="s")
            rm2 = smallp.tile([P, 2], F32, tag="rm2")
            for h in range(2):
                rh = r_sb[:, h * IH2:(h + 1) * IH2]
                nc.scalar.activation(out=rh, in_=halves[h], func=AF.Relu)
                nc.gpsimd.tensor_tensor(out=s_sb[:, h * IH2:(h + 1) * IH2],
                                        in0=rh, in1=rh, op=ALU.mult)
                nc.vector.tensor_reduce(out=rm2[:, h:h + 1], in_=rh,
                                        axis=mybir.AxisListType.X, op=ALU.max)
            mr = smallp.tile([P, 1], F32, tag="mr")
            nc.vector.tensor_reduce(out=mr, in_=rm2,
                                    axis=mybir.AxisListType.X, op=ALU.max)
            # Sabs = (max r)^2 (r >= 0)
            nc.vector.tensor_scalar(out=Sm8[:, jcol:jcol + 1], in0=mr,
                                    scalar1=mr, scalar2=None, op0=ALU.mult)
            # d = sign(g0)*127/clip(Sabs)
            sc2 = smallp.tile([P, 1], F32, tag="sc2")
            nc.vector.tensor_scalar(out=sc2, in0=Sm8[:, jcol:jcol + 1],
                                    scalar1=1e-30, scalar2=isg,
                                    op0=ALU.max, op1=ALU.mult)
            dr = smallp.tile([P, 1], F32, tag="dr")
            nc.vector.reciprocal(out=dr, in_=sc2)
            return s_sb, dr

        def down_b(s_sb, dr, q28, jcol, o_tiles):
            """Quantize, transpose, down mm, o drain."""
            rt = rtp.tile([P, I], F32, tag="rt")
            nc.vector.tensor_scalar(out=rt, in0=s_sb, scalar1=dr,
                                    scalar2=MAGIC, op0=ALU.mult, op1=ALU.add)
            iu = iup.tile([P, I], BF16, tag="iu")
            nc.vector.tensor_scalar(out=iu, in0=rt, scalar1=MAGIC,
                                    scalar2=None, op0=ALU.subtract)
            # q2 = sum(iu^2) via ACT square with accumulator
            junk = junkp.tile([P, I], BF16, tag="junk")
            nc.scalar.activation(out=junk, in_=iu, func=AF.Square,
                                 accum_out=q28[:, jcol:jcol + 1])
            # transpose iu; copy to SBUF in halves so the first down
            # matmuls can start while the second half still transposes
            iuT_ps = ps_iuT.tile([P, NKI, P], BF16, tag="iuT")
            iuT_sb = iuTp.tile([P, NKI, P], BF16, tag="iuTsb")
            NH = NKI // 2
            for h in range(2):
                for k in range(h * NH, (h + 1) * NH):
                    nc.tensor.transpose(out=iuT_ps[:, k, :],
                                        in_=iu[:, k * P:(k + 1) * P],
                                        identity=ident)
                nc.vector.tensor_copy(out=iuT_sb[:, h * NH:(h + 1) * NH, :],
                                      in_=iuT_ps[:, h * NH:(h + 1) * NH, :])
            # down matmul
            o_ps = ps_o.tile([P, H], F32, tag="o")
            for k in range(NKI):
                nc.tensor.matmul(out=o_ps, lhsT=iuT_sb[:, k, :],
                                 rhs=wdn_q[k],
                                 start=(k == 0), stop=(k == NKI - 1))
            o_sb = outp.tile([P, H], F32, tag="osb")
            nc.scalar.copy(out=o_sb, in_=o_ps)
            o_tiles.append(o_sb)

        def batch_beta_store(ib, Sm8, q28, t08, o_tiles):
            """Batched per-token output scale beta, then scale + store.
            The pure scale/square steps run on ACT (it has slack); the
            tensor*tensor steps stay on DVE."""
            scc8 = batchp.tile([P, BG], F32, tag="scc8")
            nc.vector.tensor_scalar_max(out=scc8, in0=Sm8, scalar1=1e-30)
            ga8 = batchp.tile([P, BG], F32, tag="ga8")
            nc.scalar.activation(out=ga8, in_=t08, func=AF.Identity,
                                 scale=k1b)
            al8 = batchp.tile([P, BG], F32, tag="al8")
            nc.scalar.activation(out=al8, in_=ga8, func=AF.Square)
            m18 = batchp.tile([P, BG], F32, tag="m18")
            nc.vector.tensor_tensor(out=m18, in0=al8, in1=scc8, op=ALU.mult)
            m28 = batchp.tile([P, BG], F32, tag="m28")
            nc.scalar.activation(out=m28, in_=m18, func=AF.Square)
            v18 = batchp.tile([P, BG], F32, tag="v18")
            nc.vector.tensor_tensor(out=v18, in0=m28, in1=q28, op=ALU.mult)
            Ve8 = batchp.tile([P, BG], F32, tag="Ve8")
            nc.vector.tensor_scalar(out=Ve8, in0=v18, scalar1=KV,
                                    scalar2=EPS, op0=ALU.mult, op1=ALU.add)
            sq8 = batchp.tile([P, BG], F32, tag="sq8")
            nc.scalar.activation(out=sq8, in_=Ve8, func=AF.Sqrt)
            cr8 = batchp.tile([P, BG], F32, tag="cr8")
            nc.vector.reciprocal(out=cr8, in_=sq8)
            h18 = batchp.tile([P, BG], F32, tag="h18")
            nc.scalar.activation(out=h18, in_=cr8, func=AF.Square)
            h28 = batchp.tile([P, BG], F32, tag="h28")
            nc.vector.tensor_tensor(out=h28, in0=h18, in1=Ve8, op=ALU.mult)
            h38 = batchp.tile([P, BG], F32, tag="h38")
            nc.vector.tensor_scalar(out=h38, in0=h28, scalar1=-0.5,
                                    scalar2=1.5, op0=ALU.mult, op1=ALU.add)
            c8 = batchp.tile([P, BG], F32, tag="c8")
            nc.vector.tensor_tensor(out=c8, in0=cr8, in1=h38, op=ALU.mult)
            m1g8 = batchp.tile([P, BG], F32, tag="m1g8")
            nc.scalar.activation(out=m1g8, in_=m18, func=AF.Identity,
                                 scale=g0a)
            mu8 = batchp.tile([P, BG], F32, tag="mu8")
            nc.vector.tensor_tensor(out=mu8, in0=c8, in1=m1g8, op=ALU.mult)
            b8 = batchp.tile([P, BG], F32, tag="b8")
            nc.vector.tensor_scalar(out=b8, in0=mu8, scalar1=1e-5,
                                    scalar2=wdk, op0=ALU.max, op1=ALU.mult)
            for j in range(BG):
                r0 = (ib + j) * P
                o2 = o2p.tile([P, H], F32, tag="o2")
                nc.scalar.activation(out=o2, in_=o_tiles[j],
                                     func=AF.Identity,
                                     scale=b8[:, j:j + 1])
                nc.sync.dma_start(out=out_d[r0:r0 + P, :], in_=o2)

        # -------- software-pipelined main loop (pipeline carries across
        # batch boundaries so the PE never drains between batches) --------
        pend = None   # (ih_halves, Sm8, q28, o_tiles, jcol, ib, t08)
        cur = None
        for jj in range(NT):
            ib = (jj // BG) * BG
            j = jj % BG
            if j == 0:
                x_tiles, t08, xsc8 = state.pop(ib)
                if ib + 2 * BG < NT:
                    phase_a(ib + 2 * BG)
                Sm8 = batchp.tile([P, BG], F32, tag="Sm8")
                q28 = batchp.tile([P, BG], F32, tag="q28")
                cur = dict(Sm8=Sm8, q28=q28, o_tiles=[], ib=ib, t08=t08)
            if pend is not None:
                p_ih, p_cur, p_j = pend
                s_p, dr_p = down_a(p_ih, p_cur["Sm8"], p_j)
            xT_sb = up_a(x_tiles[j], xsc8, j)
            ih = up_b(xT_sb)
            if pend is not None:
                down_b(s_p, dr_p, p_cur["q28"], p_j, p_cur["o_tiles"])
                if p_j == BG - 1:
                    batch_beta_store(p_cur["ib"], p_cur["Sm8"],
                                     p_cur["q28"], p_cur["t08"],
                                     p_cur["o_tiles"])
            pend = (ih, cur, j)
        # drain the last tile
        p_ih, p_cur, p_j = pend
        s_p, dr_p = down_a(p_ih, p_cur["Sm8"], p_j)
        down_b(s_p, dr_p, p_cur["q28"], p_j, p_cur["o_tiles"])
        batch_beta_store(p_cur["ib"], p_cur["Sm8"], p_cur["q28"],
                         p_cur["t08"], p_cur["o_tiles"])

    _split_sync_waits(nc)
    return nc


# ===========================================================================
# Fallback build for non-constant g (not exercised by the harness inputs,
# kept for correctness): original implementation.
# ===========================================================================

def _emit_weight_quant_g(nc, stage, junkp, ps, consts, wT_dram, n_ktiles,
                         nsub, name, magicb):
    n_elem = n_ktiles * 128 * nsub * 512
    asum = consts.tile([P, n_ktiles], F32, tag=f"{name}_asum")
    for k in range(n_ktiles):
        wf = stage.tile([P, nsub * 512], F32, tag="stage")
        nc.gpsimd.dma_start(out=wf, in_=wT_dram[k * P:(k + 1) * P, :])
        junk = junkp.tile([P, nsub * 512], BF16, tag="junk")
        nc.scalar.activation(out=junk, in_=wf, func=AF.Abs,
                             accum_out=asum[:, k:k + 1])
    tot = consts.tile([P, 1], F32, tag=f"{name}_tot")
    nc.vector.tensor_reduce(out=tot, in_=asum, axis=mybir.AxisListType.X,
                            op=ALU.add)
    ones128 = stage.tile([P, P], F32, tag="ones128")
    nc.vector.memset(ones128, 1.0)
    totp = ps.tile([P, 1], F32, tag="totp")
    nc.tensor.matmul(out=totp, lhsT=ones128, rhs=tot, start=True, stop=True)
    gsum = consts.tile([P, 1], F32, tag=f"{name}_gsum")
    nc.scalar.copy(out=gsum, in_=totp)
    meanclip = consts.tile([P, 1], F32, tag=f"{name}_meanclip")
    nc.vector.tensor_scalar(out=meanclip, in0=gsum, scalar1=1.0 / n_elem,
                            scalar2=1e-5, op0=ALU.mult, op1=ALU.max)
    swq = consts.tile([P, 1], F32, tag=f"{name}_swq")
    nc.vector.reciprocal(out=swq, in_=meanclip)

    wq_tiles = []
    for k in range(n_ktiles):
        wf = stage.tile([P, nsub * 512], F32, tag="stage")
        nc.gpsimd.dma_start(out=wf, in_=wT_dram[k * P:(k + 1) * P, :])
        rt = stage.tile([P, nsub * 512], F32, tag="stage_rt")
        nc.scalar.activation(out=rt, in_=wf, func=AF.Identity,
                             bias=magicb, scale=swq)
        cl = stage.tile([P, nsub * 512], F32, tag="stage_cl")
        nc.vector.tensor_scalar(out=cl, in0=rt, scalar1=MAGIC, scalar2=1.0,
                                op0=ALU.subtract, op1=ALU.min)
        wq = consts.tile([P, nsub, 512], BF16, tag=f"{name}_wq{k}")
        nc.vector.tensor_scalar(out=wq.rearrange("p a b -> p (a b)"), in0=cl,
                                scalar1=-1.0, scalar2=None, op0=ALU.max)
        wq_tiles.append(wq)
    return wq_tiles, meanclip


def build_nc_general():
    nc = bass.Bass()
    x_d = nc.dram_tensor("x", [TPC, H], F32, kind="ExternalInput")
    wupT_d = nc.dram_tensor("wupT", [H, I], F32, kind="ExternalInput")
    wdnT_d = nc.dram_tensor("wdnT", [I, H], F32, kind="ExternalInput")
    g_d = nc.dram_tensor("g", [I], F32, kind="ExternalInput")
    out_d = nc.dram_tensor("out", [TPC, H], F32, kind="ExternalOutput")

    from contextlib import ExitStack
    with ExitStack() as ctx:
        tc = ctx.enter_context(tile.TileContext(nc))
        consts = ctx.enter_context(tc.tile_pool(name="consts", bufs=1))

        ident = consts.tile([P, P], BF16)
        make_identity(nc, ident)
        magicb = consts.tile([P, 1], F32)
        nc.vector.memset(magicb, MAGIC)

        g_bc = consts.tile([P, I], F32)
        g_ap = g_d[:]
        g_bcast_ap = bass.AP(tensor=g_ap.tensor, offset=g_ap.offset,
                             ap=[[0, P]] + list(g_ap.ap))
        nc.gpsimd.dma_start(out=g_bc, in_=g_bcast_ap)

        with tc.tile_pool(name="wstage", bufs=2) as stage, \
                tc.tile_pool(name="wjunk", bufs=2) as junkp, \
                tc.tile_pool(name="wps", bufs=1, space="PSUM") as wps:
            wup_q, up_meanclip = _emit_weight_quant_g(
                nc, stage, junkp, wps, consts, wupT_d, NKH, NB, "wup", magicb)
            wdn_q, dn_meanclip = _emit_weight_quant_g(
                nc, stage, junkp, wps, consts, wdnT_d, NKI, 1, "wdn", magicb)

        k1b = consts.tile([P, 1], F32)
        nc.vector.tensor_scalar_mul(out=k1b, in0=up_meanclip,
                                    scalar1=1.0 / 127.0)
        wdk = consts.tile([P, 1], F32)
        nc.vector.tensor_scalar_mul(out=wdk, in0=dn_meanclip,
                                    scalar1=1.0 / 127.0)
        isg = consts.tile([P, 1], F32)
        nc.vector.memset(isg, 1.0 / 127.0)

        BG = 8
        xs_pool = ctx.enter_context(tc.tile_pool(name="xs", bufs=2 * BG))
        xq_pool = ctx.enter_context(tc.tile_pool(name="xqp", bufs=3))
        big = ctx.enter_context(tc.tile_pool(name="big", bufs=2))
        iup = ctx.enter_context(tc.tile_pool(name="iup", bufs=3))
        outp = ctx.enter_context(tc.tile_pool(name="outp", bufs=BG + 1))
        o2p = ctx.enter_context(tc.tile_pool(name="o2p", bufs=3))
        junkp = ctx.enter_context(tc.tile_pool(name="mjunk", bufs=1))
        small = ctx.enter_context(tc.tile_pool(name="small", bufs=3))
        batchp = ctx.enter_context(tc.tile_pool(name="batchp", bufs=2))
        ps_xT = ctx.enter_context(tc.tile_pool(name="ps_xT", bufs=1,
                                               space="PSUM"))
        ps_ih = ctx.enter_context(tc.tile_pool(name="ps_ih", bufs=1,
                                               space="PSUM"))
        ps_iuT = ctx.enter_context(tc.tile_pool(name="ps_iuT", bufs=1,
                                                space="PSUM"))
        ps_o = ctx.enter_context(tc.tile_pool(name="ps_o", bufs=1,
                                              space="PSUM"))

        IH2 = I // 2
        state = {}

        def phase_a(ib):
            xm8 = batchp.tile([P, BG], F32, tag="xm8")
            x_tiles = []
            for j in range(BG):
                r0 = (ib + j) * P
                x_sb = xs_pool.tile([P, H], F32, tag="x")
                nc.sync.dma_start(out=x_sb, in_=x_d[r0:r0 + P, :])
                x_tiles.append(x_sb)
                nc.vector.tensor_reduce(out=xm8[:, j:j + 1], in_=x_sb,
                                        axis=mybir.AxisListType.X, op=ALU.max,
                                        apply_absolute_value=True)
            t08 = batchp.tile([P, BG], F32, tag="t08")
            nc.vector.tensor_scalar_max(out=t08, in0=xm8, scalar1=1e-5)
            xr8 = batchp.tile([P, BG], F32, tag="xr8")
            nc.vector.reciprocal(out=xr8, in_=t08)
            xsc8 = batchp.tile([P, BG], F32, tag="xsc8")
            nc.vector.tensor_scalar_mul(out=xsc8, in0=xr8, scalar1=127.0)
            state[ib] = (x_tiles, t08, xsc8)

        def phase_bc(ib):
            x_tiles, t08, xsc8 = state.pop(ib)
            Sm8 = batchp.tile([P, BG], F32, tag="Sm8")
            q2g8 = batchp.tile([P, BG], F32, tag="q2g8")
            o_tiles = []

            for j in range(BG):
                x_sb = x_tiles[j]
                xq = xq_pool.tile([P, H], F32, tag="xq")
                nc.scalar.activation(out=xq, in_=x_sb, func=AF.Identity,
                                     bias=magicb, scale=xsc8[:, j:j + 1])
                ix = xq_pool.tile([P, H], BF16, tag="ix")
                nc.vector.tensor_scalar(out=ix, in0=xq, scalar1=MAGIC,
                                        scalar2=None, op0=ALU.subtract)
                xT_ps = ps_xT.tile([P, NKH, P], BF16, tag="xT")
                for k in range(NKH):
                    nc.tensor.transpose(out=xT_ps[:, k, :],
                                        in_=ix[:, k * P:(k + 1) * P],
                                        identity=ident)
                xT_sb = xq_pool.tile([P, NKH, P], BF16, tag="xTsb")
                nc.scalar.copy(out=xT_sb, in_=xT_ps)

                r_sb = big.tile([P, I], F32, tag="r")
                for h in range(2):
                    ihh = ps_ih.tile([P, IH2], F32, tag="ih")
                    for nb in range(2):
                        lo = nb * 512
                        for k in range(NKH):
                            nc.tensor.matmul(
                                out=ihh[:, lo:lo + 512],
                                lhsT=xT_sb[:, k, :],
                                rhs=wup_q[k][:, 2 * h + nb, :],
                                start=(k == 0), stop=(k == NKH - 1))
                    nc.scalar.activation(out=r_sb[:, h * IH2:(h + 1) * IH2],
                                         in_=ihh, func=AF.Relu)

                s_sb = big.tile([P, I], F32, tag="s")
                nc.gpsimd.tensor_tensor(out=s_sb, in0=r_sb, in1=r_sb,
                                        op=ALU.mult)
                sq_in = big.tile([P, I], F32, tag="sg")
                nc.vector.tensor_tensor(out=sq_in, in0=s_sb, in1=g_bc,
                                        op=ALU.mult)
                junk3 = junkp.tile([P, I], BF16, tag="junk3")
                nc.scalar.activation(out=junk3, in_=s_sb, func=AF.Square,
                                     accum_out=q2g8[:, j:j + 1])
                nc.vector.tensor_reduce(out=Sm8[:, j:j + 1], in_=sq_in,
                                        axis=mybir.AxisListType.X,
                                        op=ALU.max,
                                        apply_absolute_value=True)
                sc2 = small.tile([P, 1], F32, tag="sc2")
                nc.vector.tensor_scalar(out=sc2, in0=Sm8[:, j:j + 1],
                                        scalar1=1e-30, scalar2=isg,
                                        op0=ALU.max, op1=ALU.mult)
                dr = small.tile([P, 1], F32, tag="dr")
                nc.vector.reciprocal(out=dr, in_=sc2)
                rt = big.tile([P, I], F32, tag="rt")
                nc.vector.tensor_scalar(out=rt, in0=sq_in, scalar1=dr,
                                        scalar2=MAGIC, op0=ALU.mult,
                                        op1=ALU.add)
                iu = iup.tile([P, I], BF16, tag="iu")
                nc.vector.tensor_scalar(out=iu, in0=rt, scalar1=MAGIC,
                                        scalar2=None, op0=ALU.subtract)

                iuT_ps = ps_iuT.tile([P, NKI, P], BF16, tag="iuT")
                for k in range(NKI):
                    nc.tensor.transpose(out=iuT_ps[:, k, :],
                                        in_=iu[:, k * P:(k + 1) * P],
                                        identity=ident)
                iuT_sb = iup.tile([P, NKI, P], BF16, tag="iuTsb")
                nc.scalar.copy(out=iuT_sb, in_=iuT_ps)

                o_ps = ps_o.tile([P, H], F32, tag="o")
                for k in range(NKI):
                    nc.tensor.matmul(out=o_ps, lhsT=iuT_sb[:, k, :],
                                     rhs=wdn_q[k][:, 0, :],
                                     start=(k == 0), stop=(k == NKI - 1))
                o_sb = outp.tile([P, H], F32, tag="osb")
                nc.scalar.copy(out=o_sb, in_=o_ps)
                o_tiles.append(o_sb)

            scc8 = batchp.tile([P, BG], F32, tag="scc8")
            nc.vector.tensor_scalar_max(out=scc8, in0=Sm8, scalar1=1e-30)
            ga8 = batchp.tile([P, BG], F32, tag="ga8")
            nc.vector.tensor_scalar_mul(out=ga8, in0=t08, scalar1=k1b)
            al8 = batchp.tile([P, BG], F32, tag="al8")
            nc.vector.tensor_tensor(out=al8, in0=ga8, in1=ga8, op=ALU.mult)
            m18 = batchp.tile([P, BG], F32, tag="m18")
            nc.vector.tensor_tensor(out=m18, in0=al8, in1=scc8, op=ALU.mult)
            v18 = batchp.tile([P, BG], F32, tag="v18")
            Ve8 = batchp.tile([P, BG], F32, tag="Ve8")
            al28 = batchp.tile([P, BG], F32, tag="al28")
            nc.vector.tensor_tensor(out=al28, in0=al8, in1=al8, op=ALU.mult)
            nc.vector.tensor_tensor(out=v18, in0=al28, in1=q2g8, op=ALU.mult)
            nc.vector.tensor_scalar(out=Ve8, in0=v18, scalar1=1.0 / I,
                                    scalar2=EPS, op0=ALU.mult, op1=ALU.add)
            sq8 = batchp.tile([P, BG], F32, tag="sq8")
            nc.scalar.activation(out=sq8, in_=Ve8, func=AF.Sqrt)
            cr8 = batchp.tile([P, BG], F32, tag="cr8")
            nc.vector.reciprocal(out=cr8, in_=sq8)
            h18 = batchp.tile([P, BG], F32, tag="h18")
            nc.vector.tensor_tensor(out=h18, in0=cr8, in1=cr8, op=ALU.mult)
            h28 = batchp.tile([P, BG], F32, tag="h28")
            nc.vector.tensor_tensor(out=h28, in0=h18, in1=Ve8, op=ALU.mult)
            h38 = batchp.tile([P, BG], F32, tag="h38")
            nc.vector.tensor_scalar(out=h38, in0=h28, scalar1=-0.5,
                                    scalar2=1.5, op0=ALU.mult, op1=ALU.add)
            c8 = batchp.tile([P, BG], F32, tag="c8")
            nc.vector.tensor_tensor(out=c8, in0=cr8, in1=h38, op=ALU.mult)
            mu8 = batchp.tile([P, BG], F32, tag="mu8")
            nc.vector.tensor_tensor(out=mu8, in0=c8, in1=m18, op=ALU.mult)
            b8 = batchp.tile([P, BG], F32, tag="b8")
            nc.vector.tensor_scalar(out=b8, in0=mu8, scalar1=1e-5,
                                    scalar2=wdk, op0=ALU.max, op1=ALU.mult)

            for j in range(BG):
                r0 = (ib + j) * P
                o2 = o2p.tile([P, H], F32, tag="o2")
                nc.vector.tensor_scalar_mul(out=o2, in0=o_tiles[j],
                                            scalar1=b8[:, j:j + 1])
                nc.sync.dma_start(out=out_d[r0:r0 + P, :], in_=o2)

        phase_a(0)
        for ib in range(0, NT, BG):
            if ib + BG < NT:
                phase_a(ib + BG)
            phase_bc(ib)

    _split_sync_waits(nc)
    return nc


_NC_CACHE = {}


def kernel(x, w_up, w_down, g):
    global LAST_RESULT
    x = np.ascontiguousarray(x, dtype=np.float32)
    w_up = np.ascontiguousarray(w_up, dtype=np.float32)
    w_down = np.ascontiguousarray(w_down, dtype=np.float32)
    g = np.ascontiguousarray(g, dtype=np.float32)

    if abs(float(g[0])) < 1e-30 and np.all(g == g[0]):
        return np.zeros_like(x)

    general = not bool(np.all(g == g[0]))
    key = ("gen" if general else "const")
    if key not in _NC_CACHE:
        _NC_CACHE[key] = (build_nc_general() if general
                          else build_nc_const())
    nc = _NC_CACHE[key]

    xt = x.reshape(TOK, H)
    wupT = np.ascontiguousarray(w_up.T)    # [H, I]
    wdnT = np.ascontiguousarray(w_down.T)  # [I, H]
    in_maps = [
        {"x": xt[c * TPC:(c + 1) * TPC], "wupT": wupT, "wdnT": wdnT, "g": g}
        for c in range(N_CORES)
    ]
    res = run_bass_kernel_spmd(
        nc, in_maps, list(range(N_CORES)),
        trace=bool(os.environ.get("BASS_TRACE")),
    )
    LAST_RESULT = res
    out = np.concatenate([res.results[c]["out"] for c in range(N_CORES)],
                         axis=0)
    return out.reshape(B, S, H)


# revision 39
# speedup vs baseline: 1.1499x; 1.1188x over previous
"""BitNet MLP (act_quant -> ternary matmul -> relu^2 -> SubLN -> act_quant ->
ternary matmul) on 8 Trainium2 NeuronCores, data-parallel over tokens.

Math notes (exactness):
- act_quant int levels (|q| <= 127) and ternary weights {-1,0,1} are exactly
  representable in bf16, so both matmuls run on the PE in bf16 with exact
  integer arithmetic (f32 PSUM accumulation, |sums| < 2^24).
- All quantization scales are folded into per-token scalars applied to the
  final [tok, 512] output: out = i2 * beta_t with
    beta_t = clip(c_t * alpha_t * Sabs_t, 1e-5) * clip(mean|w_dn|,1e-5) / 127
  where alpha_t = (clip(max|x_t|,1e-5) * clip(mean|w_up|,1e-5) / 127)^2,
  Sabs_t = max_i |relu(ih)^2 * g|, c_t = rsqrt(var_t + 1e-6).
- Rounding uses the magic-number trick (x + 1.5*2^23 - 1.5*2^23) == RNE
  round-to-integer for |x| < 2^22, matching jnp.round (half-to-even).
- SubLN variance is recovered from the quantized intermediate:
  var = alpha^2 * sum(iu^2) * (Sabs/127)^2 / (2048 * g0^2); the
  quantization error on sum(iu^2) is ~0.1% which is far below tolerance.
  (For non-constant g an extra pass computes sum((relu^2)^2) directly.)
"""
import os
import numpy as np

import concourse.bass as bass
import concourse.tile as tile
from concourse import mybir
from concourse.bass_utils import run_bass_kernel_spmd
from concourse.masks import make_identity

# ---------------------------------------------------------------------------
# Workaround for walrus "Too many sync wait commands" on the TileContext tail
# drain: split the drain's semaphore waits across single-wait SP NOPs, then
# advance the observed clocks so the real drain needs none.
import re as _re
import bass_rust as _bass_rust


def _patched_drain_and_barrier(self, tick_clock, wait_clock):
    gc = tick_clock.global_clock
    ticks = list(map(int, _re.findall(r"\d+", repr(gc))))
    n = len(ticks)
    nonzero = [(i, t) for i, t in enumerate(ticks) if t > 0]
    for i, t in nonzero:
        sub = [0] * n
        sub[i] = t
        sub_scoped = _bass_rust.ScopedClock({None: _bass_rust.VectorClock(sub)})
        nop = self.nc.sync.nop()
        wait_clock.add_sem_waits(nop.ins, sub_scoped)
        for ec in wait_clock.engine_clocks:
            ec.update_past(sub_scoped)
    drain_inst = self.nc.sync.drain()
    wait_clock.add_sem_waits(drain_inst.ins,
                             _bass_rust.ScopedClock({None: gc}))
    self.nc.all_engine_barrier()
    popped = self.nc._tile_sem_poison_stack.pop()
    assert popped is self._sem_poison
    self.nc.clear_and_free_semaphores(list(self.sems.allocated().values()))
    self.nc.all_engine_barrier()


tile.TileContext._drain_and_barrier = _patched_drain_and_barrier


def _split_sync_waits(nc, keep_default=1):
    """walrus caps the number of semaphore waits a single instruction can
    carry (CTRL ops take only 1; compute ops a few). Hoist excess waits onto
    single-wait NOPs inserted immediately before the instruction on the same
    engine — identical semantics, engines execute in order."""
    import dataclasses
    keep_by_op = {}
    proto = None
    for f in nc.m.functions:
        for bb in f.blocks:
            for inst in bb.instructions:
                if type(inst).__name__ == "InstNoOp":
                    proto = inst
                    break
            if proto is not None:
                break
        if proto is not None:
            break
    counter = [0]
    for f in nc.m.functions:
        new_blocks = []
        for bb in f.blocks:
            out = []
            changed = False
            for inst in bb.instructions:
                si = inst.sync_info
                ow = list(si.on_wait) if si is not None and si.on_wait else []
                keep = keep_by_op.get(inst.opcode, keep_default)
                if len(ow) > keep:
                    assert proto is not None, "no NoOp prototype found yet"
                    for w in ow[:-keep]:
                        counter[0] += 1
                        nop = dataclasses.replace(
                            proto,
                            name=f"I-waitsplit-{counter[0]}",
                            engine=inst.engine,
                            sync_info=_bass_rust.SyncInfo(on_wait=[w],
                                                          on_update=[]),
                        )
                        out.append(nop)
                    si.on_wait = ow[-keep:]
                    changed = True
                out.append(inst)
            if changed:
                bb2 = _bass_rust.BasicBlock(name=bb.name, instructions=out)
                bb2.IsExit = bb.IsExit
                bb2.IsLoopEntry = bb.IsLoopEntry
                bb2.IsPredicated = bb.IsPredicated
                new_blocks.append(bb2)
            else:
                new_blocks.append(bb)
        f.blocks = new_blocks
# ---------------------------------------------------------------------------

F32 = mybir.dt.float32
BF16 = mybir.dt.bfloat16
ALU = mybir.AluOpType
AF = mybir.ActivationFunctionType

N_CORES = 8
B, S, H, I = 8, 8192, 512, 2048
TOK = B * S                  # 65536 tokens total
TPC = TOK // N_CORES         # 8192 tokens per core
P = 128                      # partition tile
NT = TPC // P                # 64 token tiles per core
NKH = H // P                 # 4 k-tiles over H
NKI = I // P                 # 16 k-tiles over I
NB = I // 512                # 4 psum banks for the up matmul

MAGIC = 12582912.0           # 1.5 * 2^23: RNE round-to-int trick
EPS = 1e-6                   # SubLN eps (from reference)

LAST_RESULT = None           # set by kernel() for test harness introspection


# ===========================================================================
# Optimized build (constant g, the case the harness exercises)
# ===========================================================================

_DMA_QUEUES = None


def _dma_queue(nc, i):
    """Round-robin weight-prep DMAs across engine queues so the transfers
    overlap instead of serializing on one ring."""
    return [nc.gpsimd, nc.scalar, nc.sync][i % 3]


def _wide_ap(wT_dram, g, nb, width):
    """AP view [128, nb, width] packing nb consecutive 128-row k-tiles of a
    [rows, width] DRAM tensor side by side (partition p, block b ->
    row g*nb*128 + b*128 + p)."""
    base = wT_dram[:, :]
    return bass.AP(tensor=base.tensor,
                   offset=base.offset + g * nb * P * width,
                   ap=[[width, P], [P * width, nb], [1, width]])


def _prep_weight_pass1(nc, stage, wT_dram, n_ktiles, width, name, consts,
                       nb=1):
    """DMA the weight tiles (nb k-tiles per transfer) and accumulate
    per-partition |w| sums (DVE)."""
    ng = n_ktiles // nb
    asum = consts.tile([P, ng], F32, tag=f"{name}_asum")
    for g in range(ng):
        wf = stage.tile([P, nb * width], F32, tag="wst")
        _dma_queue(nc, g).dma_start(out=wf, in_=_wide_ap(wT_dram, g, nb,
                                                         width))
        nc.vector.tensor_reduce(out=asum[:, g:g + 1], in_=wf,
                                axis=mybir.AxisListType.X, op=ALU.add,
                                apply_absolute_value=True)
    return asum


def _prep_weight_scale(nc, stage, ps, consts, asum, n_elem, name):
    """Reduce per-partition sums to the global clip(mean|w|,1e-5) broadcast
    to all partitions, plus its reciprocal."""
    tot = consts.tile([P, 1], F32, tag=f"{name}_tot")
    nc.vector.tensor_reduce(out=tot, in_=asum, axis=mybir.AxisListType.X,
                            op=ALU.add)
    ones128 = stage.tile([P, P], F32, tag=f"{name}_ones")
    nc.vector.memset(ones128, 1.0)
    totp = ps.tile([P, 1], F32, tag=f"{name}_totp")
    nc.tensor.matmul(out=totp, lhsT=ones128, rhs=tot, start=True, stop=True)
    gsum = consts.tile([P, 1], F32, tag=f"{name}_gsum")
    nc.scalar.copy(out=gsum, in_=totp)
    meanclip = consts.tile([P, 1], F32, tag=f"{name}_meanclip")
    nc.vector.tensor_scalar(out=meanclip, in0=gsum, scalar1=1.0 / n_elem,
                            scalar2=1e-5, op0=ALU.mult, op1=ALU.max)
    swq = consts.tile([P, 1], F32, tag=f"{name}_swq")
    nc.vector.reciprocal(out=swq, in_=meanclip)
    return meanclip, swq


def _prep_weight_pass2_tile(nc, stage, consts, wT_dram, k, width, name,
                            magicb, swq, out_dt, nb=1):
    """Re-DMA nb k-tiles and quantize to ternary in dtype out_dt.
    Returns a [P, nb*width] tile."""
    wf = stage.tile([P, nb * width], F32, tag="wst")
    if nb == 1:
        _dma_queue(nc, k).dma_start(out=wf,
                                    in_=wT_dram[k * P:(k + 1) * P, :])
    else:
        _dma_queue(nc, k).dma_start(out=wf, in_=_wide_ap(wT_dram, k, nb,
                                                         width))
    width = nb * width
    rt = stage.tile([P, width], F32, tag="wrt")
    nc.scalar.activation(out=rt, in_=wf, func=AF.Identity,
                         bias=magicb, scale=swq)
    nc.vector.tensor_scalar(out=rt, in0=rt, scalar1=MAGIC, scalar2=1.0,
                            op0=ALU.subtract, op1=ALU.min)
    wq = consts.tile([P, width], out_dt, tag=f"{name}_wq{k}")
    nc.vector.tensor_scalar(out=wq, in0=rt, scalar1=-1.0, scalar2=None,
                            op0=ALU.max)
    return wq


def build_nc_const():
    nc = bass.Bass()
    x_d = nc.dram_tensor("x", [TPC, H], F32, kind="ExternalInput")
    wupT_d = nc.dram_tensor("wupT", [H, I], F32, kind="ExternalInput")
    wdnT_d = nc.dram_tensor("wdnT", [I, H], F32, kind="ExternalInput")
    g_d = nc.dram_tensor("g", [I], F32, kind="ExternalInput")
    out_d = nc.dram_tensor("out", [TPC, H], F32, kind="ExternalOutput")

    BG = 8                    # tiles per small-op batch
    KV = 1.0 / (127.0 * 127.0 * I)

    from contextlib import ExitStack
    with ExitStack() as ctx:
        tc = ctx.enter_context(tile.TileContext(nc))

        consts = ctx.enter_context(tc.tile_pool(name="consts", bufs=1))

        ident = consts.tile([P, P], BF16)
        make_identity(nc, ident)
        magicb = consts.tile([P, 1], F32)
        nc.vector.memset(magicb, MAGIC)
        negmb = consts.tile([P, 1], F32)
        nc.vector.memset(negmb, -MAGIC)

        # -------- main-loop pools (created before prep so x prefetch can
        # land while weights stream in) --------
        xs_pool = ctx.enter_context(tc.tile_pool(name="xs", bufs=2 * BG))
        xq_pool = ctx.enter_context(tc.tile_pool(name="xqp", bufs=2))
        hlp = ctx.enter_context(tc.tile_pool(name="hlp", bufs=2))
        xTp = ctx.enter_context(tc.tile_pool(name="xTp", bufs=2))
        rp = ctx.enter_context(tc.tile_pool(name="rp", bufs=2))
        rtp = ctx.enter_context(tc.tile_pool(name="rtp", bufs=2))
        iup = ctx.enter_context(tc.tile_pool(name="iup", bufs=2))
        iuTp = ctx.enter_context(tc.tile_pool(name="iuTp", bufs=2))
        junkp = ctx.enter_context(tc.tile_pool(name="junkp", bufs=1))
        smallp = ctx.enter_context(tc.tile_pool(name="smallp", bufs=4))
        outp = ctx.enter_context(tc.tile_pool(name="outp", bufs=BG + 1))
        o2p = ctx.enter_context(tc.tile_pool(name="o2p", bufs=3))
        batchp = ctx.enter_context(tc.tile_pool(name="batchp", bufs=2))

        # -------- x prefetch state --------
        state = {}

        def phase_a(ib):
            """DMA x tiles + per-token absmax, then batched x-scale chain."""
            xm8 = batchp.tile([P, BG], F32, tag="xm8")
            x_tiles = []
            for j in range(BG):
                r0 = (ib + j) * P
                x_sb = xs_pool.tile([P, H], F32, tag="x")
                nc.sync.dma_start(out=x_sb, in_=x_d[r0:r0 + P, :])
                x_tiles.append(x_sb)
                nc.vector.tensor_reduce(out=xm8[:, j:j + 1], in_=x_sb,
                                        axis=mybir.AxisListType.X, op=ALU.max,
                                        apply_absolute_value=True)
            t08 = batchp.tile([P, BG], F32, tag="t08")
            nc.vector.tensor_scalar_max(out=t08, in0=xm8, scalar1=1e-5)
            xr8 = batchp.tile([P, BG], F32, tag="xr8")
            nc.vector.reciprocal(out=xr8, in_=t08)
            xsc8 = batchp.tile([P, BG], F32, tag="xsc8")
            nc.vector.tensor_scalar_mul(out=xsc8, in0=xr8, scalar1=127.0)
            state[ib] = (x_tiles, t08, xsc8)

        # issue the first two batches of x loads before weight prep so the
        # DMA and absmax work overlaps the weight streaming
        phase_a(0)
        phase_a(BG)

        # -------- weight prep --------
        with tc.tile_pool(name="wstage", bufs=2) as stage, \
                tc.tile_pool(name="wps", bufs=1, space="PSUM") as wps:
            # g0 broadcast [128,1] via K=1 matmul with ones (g is constant)
            ones_row = stage.tile([1, P], F32, tag="ones_row")
            nc.vector.memset(ones_row, 1.0)
            g0_sb = stage.tile([1, 1], F32, tag="g0sb")
            nc.gpsimd.dma_start(out=g0_sb, in_=g_d[0:1])
            g0_ps = wps.tile([P, 1], F32, tag="g0ps")
            nc.tensor.matmul(out=g0_ps, lhsT=ones_row, rhs=g0_sb, start=True,
                             stop=True)
            g0b = consts.tile([P, 1], F32)
            nc.scalar.copy(out=g0b, in_=g0_ps)

            up_asum = _prep_weight_pass1(nc, stage, wupT_d, NKH, I, "wup",
                                         consts)
            dn_asum = _prep_weight_pass1(nc, stage, wdnT_d, NKI, H, "wdn",
                                         consts, nb=4)
            up_meanclip, up_swq = _prep_weight_scale(
                nc, stage, wps, consts, up_asum, H * I, "wup")
            dn_meanclip, dn_swq = _prep_weight_scale(
                nc, stage, wps, consts, dn_asum, H * I, "wdn")

            wup_q = []
            for k in range(NKH):
                wq = _prep_weight_pass2_tile(nc, stage, consts, wupT_d, k, I,
                                             "wup", magicb, up_swq, BF16)
                wup_q.append(wq.rearrange("p (a b) -> p a b", b=512))
            wdn_g = []
            for g in range(NKI // 4):
                wq = _prep_weight_pass2_tile(nc, stage, consts, wdnT_d, g, H,
                                             "wdn", magicb, dn_swq, BF16,
                                             nb=4)
                wdn_g.append(wq.rearrange("p (a b) -> p a b", b=512))
            wdn_q = [wdn_g[k // 4][:, k % 4, :] for k in range(NKI)]

        # per-token gamma multiplier and final output multiplier
        k1b = consts.tile([P, 1], F32)
        nc.vector.tensor_scalar_mul(out=k1b, in0=up_meanclip,
                                    scalar1=1.0 / 127.0)
        wdk = consts.tile([P, 1], F32)
        nc.vector.tensor_scalar_mul(out=wdk, in0=dn_meanclip,
                                    scalar1=1.0 / 127.0)
        sg127 = consts.tile([P, 1], F32)
        nc.scalar.activation(out=sg127, in_=g0b, func=AF.Sign)
        nc.vector.tensor_scalar_mul(out=sg127, in0=sg127, scalar1=127.0)
        g0a = consts.tile([P, 1], F32)
        nc.scalar.activation(out=g0a, in_=g0b, func=AF.Abs)
        # isg = sign(g0)/127: d = 1/(Sabs*isg) = sign*127/Sabs
        isg = consts.tile([P, 1], F32)
        nc.vector.tensor_scalar_mul(out=isg, in0=sg127,
                                    scalar1=1.0 / (127.0 * 127.0))

        # -------- PSUM pools --------
        ps_xT = ctx.enter_context(tc.tile_pool(name="ps_xT", bufs=1,
                                               space="PSUM"))
        ps_ih = [ctx.enter_context(tc.tile_pool(name=f"ps_ih{c}", bufs=1,
                                                space="PSUM"))
                 for c in range(2)]
        ps_iuT = ctx.enter_context(tc.tile_pool(name="ps_iuT", bufs=1,
                                                space="PSUM"))
        ps_o = ctx.enter_context(tc.tile_pool(name="ps_o", bufs=1,
                                              space="PSUM"))
        IH2 = I // 2

        def up_a1(x_sb, xsc8, jcol):
            """Quantize x to bf16 int8 (ACT only)."""
            xqm = xq_pool.tile([P, H], F32, tag="xqm")
            nc.scalar.activation(out=xqm, in_=x_sb, func=AF.Identity,
                                 bias=magicb, scale=xsc8[:, jcol:jcol + 1])
            ix = hlp.tile([P, H], BF16, tag="ix")
            nc.scalar.activation(out=ix, in_=xqm, func=AF.Identity,
                                 bias=negmb)
            return ix

        def up_a2(ix):
            """Transpose ix and stage for the next tile's up matmul."""
            xT_ps = ps_xT.tile([P, NKH, P], BF16, tag="xT")
            for k in range(NKH):
                nc.tensor.transpose(out=xT_ps[:, k, :],
                                    in_=ix[:, k * P:(k + 1) * P],
                                    identity=ident)
            xT_sb = xTp.tile([P, NKH, P], BF16, tag="xTsb")
            nc.scalar.copy(out=xT_sb, in_=xT_ps)
            return xT_sb

        def up_b(xT_sb):
            """Up matmul into 2 psum half pools. Half-outer so the first
            half's matmuls only wait on the first half's relu drain."""
            halves = [ps_ih[h].tile([P, IH2], F32, tag=f"ih{h}",
                                    name=f"ih{h}") for h in range(2)]
            for h in range(2):
                for k in range(NKH):
                    for c in (2 * h, 2 * h + 1):
                        nc.tensor.matmul(out=halves[h][:, (c % 2) * 512:
                                                       (c % 2) * 512 + 512],
                                         lhsT=xT_sb[:, k, :],
                                         rhs=wup_q[k][:, c, :],
                                         start=(k == 0),
                                         stop=(k == NKH - 1))
            return halves

        def down_a(halves, Sm8, jcol):
            """relu-drain the up psum halves to SBUF; per-half square (Pool)
            and max (DVE) start as soon as each half lands; then the
            per-token quant scale d."""
            r_sb = rp.tile([P, I], F32, tag="r")
            s_sb = rtp.tile([P, I], F32, tag="s")
            rm2 = smallp.tile([P, 2], F32, tag="rm2")
            for h in range(2):
                rh = r_sb[:, h * IH2:(h + 1) * IH2]
                nc.scalar.activation(out=rh, in_=halves[h], func=AF.Relu)
                nc.gpsimd.tensor_tensor(out=s_sb[:, h * IH2:(h + 1) * IH2],
                                        in0=rh, in1=rh, op=ALU.mult)
                nc.vector.tensor_reduce(out=rm2[:, h:h + 1], in_=rh,
                                        axis=mybir.AxisListType.X, op=ALU.max)
            mr = smallp.tile([P, 1], F32, tag="mr")
            nc.vector.tensor_reduce(out=mr, in_=rm2,
                                    axis=mybir.AxisListType.X, op=ALU.max)
            # Sabs = (max r)^2 (r >= 0)
            nc.vector.tensor_scalar(out=Sm8[:, jcol:jcol + 1], in0=mr,
                                    scalar1=mr, scalar2=None, op0=ALU.mult)
            # d = sign(g0)*127/clip(Sabs)
            sc2 = smallp.tile([P, 1], F32, tag="sc2")
            nc.vector.tensor_scalar(out=sc2, in0=Sm8[:, jcol:jcol + 1],
                                    scalar1=1e-30, scalar2=isg,
                                    op0=ALU.max, op1=ALU.mult)
            dr = smallp.tile([P, 1], F32, tag="dr")
            nc.vector.reciprocal(out=dr, in_=sc2)
            return s_sb, dr

        def down_b1(s_sb, dr, q28, jcol):
            """Quantize and transpose iu (PE transposes precede the next
            tile's up matmuls in the PE stream)."""
            rt = rtp.tile([P, I], F32, tag="rt")
            nc.vector.tensor_scalar(out=rt, in0=s_sb, scalar1=dr,
                                    scalar2=MAGIC, op0=ALU.mult, op1=ALU.add)
            iu = iup.tile([P, I], BF16, tag="iu")
            nc.vector.tensor_scalar(out=iu, in0=rt, scalar1=MAGIC,
                                    scalar2=None, op0=ALU.subtract)
            # q2 = sum(iu^2) via ACT square with accumulator
            junk = junkp.tile([P, I], BF16, tag="junk")
            nc.scalar.activation(out=junk, in_=iu, func=AF.Square,
                                 accum_out=q28[:, jcol:jcol + 1])
            # transpose iu; copy to SBUF in halves
            iuT_ps = ps_iuT.tile([P, NKI, P], BF16, tag="iuT")
            iuT_sb = iuTp.tile([P, NKI, P], BF16, tag="iuTsb")
            NH = NKI // 2
            for h in range(2):
                for k in range(h * NH, (h + 1) * NH):
                    nc.tensor.transpose(out=iuT_ps[:, k, :],
                                        in_=iu[:, k * P:(k + 1) * P],
                                        identity=ident)
                nc.vector.tensor_copy(out=iuT_sb[:, h * NH:(h + 1) * NH, :],
                                      in_=iuT_ps[:, h * NH:(h + 1) * NH, :])
            return iuT_sb

        def down_b2(iuT_sb, o_tiles):
            """Down matmul and o drain."""
            o_ps = ps_o.tile([P, H], F32, tag="o")
            for k in range(NKI):
                nc.tensor.matmul(out=o_ps, lhsT=iuT_sb[:, k, :],
                                 rhs=wdn_q[k],
                                 start=(k == 0), stop=(k == NKI - 1))
            o_sb = outp.tile([P, H], F32, tag="osb")
            nc.scalar.copy(out=o_sb, in_=o_ps)
            o_tiles.append(o_sb)

        def batch_beta_store(ib, Sm8, q28, t08, o_tiles):
            """Batched per-token output scale beta, then scale + store.
            The pure scale/square steps run on ACT (it has slack); the
            tensor*tensor steps stay on DVE."""
            scc8 = batchp.tile([P, BG], F32, tag="scc8")
            nc.vector.tensor_scalar_max(out=scc8, in0=Sm8, scalar1=1e-30)
            ga8 = batchp.tile([P, BG], F32, tag="ga8")
            nc.scalar.activation(out=ga8, in_=t08, func=AF.Identity,
                                 scale=k1b)
            al8 = batchp.tile([P, BG], F32, tag="al8")
            nc.scalar.activation(out=al8, in_=ga8, func=AF.Square)
            m18 = batchp.tile([P, BG], F32, tag="m18")
            nc.vector.tensor_tensor(out=m18, in0=al8, in1=scc8, op=ALU.mult)
            m28 = batchp.tile([P, BG], F32, tag="m28")
            nc.scalar.activation(out=m28, in_=m18, func=AF.Square)
            v18 = batchp.tile([P, BG], F32, tag="v18")
            nc.vector.tensor_tensor(out=v18, in0=m28, in1=q28, op=ALU.mult)
            Ve8 = batchp.tile([P, BG], F32, tag="Ve8")
            nc.vector.tensor_scalar(out=Ve8, in0=v18, scalar1=KV,
                                    scalar2=EPS, op0=ALU.mult, op1=ALU.add)
            sq8 = batchp.tile([P, BG], F32, tag="sq8")
            nc.scalar.activation(out=sq8, in_=Ve8, func=AF.Sqrt)
            cr8 = batchp.tile([P, BG], F32, tag="cr8")
            nc.vector.reciprocal(out=cr8, in_=sq8)
            h18 = batchp.tile([P, BG], F32, tag="h18")
            nc.scalar.activation(out=h18, in_=cr8, func=AF.Square)
            h28 = batchp.tile([P, BG], F32, tag="h28")
            nc.vector.tensor_tensor(out=h28, in0=h18, in1=Ve8, op=ALU.mult)
            h38 = batchp.tile([P, BG], F32, tag="h38")
            nc.vector.tensor_scalar(out=h38, in0=h28, scalar1=-0.5,
                                    scalar2=1.5, op0=ALU.mult, op1=ALU.add)
            c8 = batchp.tile([P, BG], F32, tag="c8")
            nc.vector.tensor_tensor(out=c8, in0=cr8, in1=h38, op=ALU.mult)
            m1g8 = batchp.tile([P, BG], F32, tag="m1g8")
            nc.scalar.activation(out=m1g8, in_=m18, func=AF.Identity,
                                 scale=g0a)
            mu8 = batchp.tile([P, BG], F32, tag="mu8")
            nc.vector.tensor_tensor(out=mu8, in0=c8, in1=m1g8, op=ALU.mult)
            b8 = batchp.tile([P, BG], F32, tag="b8")
            nc.vector.tensor_scalar(out=b8, in0=mu8, scalar1=1e-5,
                                    scalar2=wdk, op0=ALU.max, op1=ALU.mult)
            for j in range(BG):
                r0 = (ib + j) * P
                o2 = o2p.tile([P, H], F32, tag="o2")
                if j % 2 == 0:
                    nc.scalar.activation(out=o2, in_=o_tiles[j],
                                         func=AF.Identity,
                                         scale=b8[:, j:j + 1])
                else:
                    nc.vector.tensor_scalar_mul(out=o2, in0=o_tiles[j],
                                                scalar1=b8[:, j:j + 1])
                nc.sync.dma_start(out=out_d[r0:r0 + P, :], in_=o2)

        # -------- software-pipelined main loop --------
        # Depth-3 pipeline carried across batch boundaries. Per iteration t
        # the emission is:
        #   down_a(t-1) [ACT relu first] -> up_a1(t+1) [ACT quantize] ->
        #   up_b(t) [PE up mms] -> down_b1(t-1) [DVE quant + PE iuT-T] ->
        #   down_b2(t-1) [PE down mms] -> up_a2(t+1) [PE xT-T]
        # so the PE stream repeats [up-mm | iuT-T | down-mm | xT-T] with
        # every operand produced about one tile period in advance.
        bxs = {}   # ib -> (x_tiles, t08, xsc8)
        bst = {}   # ib -> batch beta state

        def get_bxs(t):
            ib = (t // BG) * BG
            if ib not in bxs:
                bxs[ib] = state.pop(ib)
                if ib + 2 * BG < NT:
                    phase_a(ib + 2 * BG)
            return bxs[ib]

        def get_bst(t):
            ib = (t // BG) * BG
            if ib not in bst:
                Sm8 = batchp.tile([P, BG], F32, tag="Sm8")
                q28 = batchp.tile([P, BG], F32, tag="q28")
                bst[ib] = dict(Sm8=Sm8, q28=q28, o_tiles=[], ib=ib,
                               t08=get_bxs(t)[1])
            return bst[ib]

        xTs = {}    # t -> staged xT_sb
        ihs = {}    # t -> up psum halves
        b0 = get_bxs(0)
        xTs[0] = up_a2(up_a1(b0[0][0], b0[2], 0))
        for t in range(NT):
            if t - 1 >= 0:
                st = get_bst(t - 1)
                s_p, dr_p = down_a(ihs.pop(t - 1), st["Sm8"], (t - 1) % BG)
            if t + 1 < NT:
                bn = get_bxs(t + 1)
                ix_n = up_a1(bn[0][(t + 1) % BG], bn[2], (t + 1) % BG)
            ihs[t] = up_b(xTs.pop(t))
            if t - 1 >= 0:
                iuT_sb = down_b1(s_p, dr_p, st["q28"], (t - 1) % BG)
                down_b2(iuT_sb, st["o_tiles"])
                if (t - 1) % BG == BG - 1:
                    batch_beta_store(st["ib"], st["Sm8"], st["q28"],
                                     st["t08"], st["o_tiles"])
                    del bst[st["ib"]]
                    del bxs[st["ib"]]
            if t + 1 < NT:
                xTs[t + 1] = up_a2(ix_n)
        # drain the last tile
        st = get_bst(NT - 1)
        s_p, dr_p = down_a(ihs.pop(NT - 1), st["Sm8"], (NT - 1) % BG)
        iuT_sb = down_b1(s_p, dr_p, st["q28"], (NT - 1) % BG)
        down_b2(iuT_sb, st["o_tiles"])
        batch_beta_store(st["ib"], st["Sm8"], st["q28"], st["t08"],
                         st["o_tiles"])

    _split_sync_waits(nc)
    return nc


# ===========================================================================
# Fallback build for non-constant g (not exercised by the harness inputs,
# kept for correctness): original implementation.
# ===========================================================================

def _emit_weight_quant_g(nc, stage, junkp, ps, consts, wT_dram, n_ktiles,
                         nsub, name, magicb):
    n_elem = n_ktiles * 128 * nsub * 512
    asum = consts.tile([P, n_ktiles], F32, tag=f"{name}_asum")
    for k in range(n_ktiles):
        wf = stage.tile([P, nsub * 512], F32, tag="stage")
        nc.gpsimd.dma_start(out=wf, in_=wT_dram[k * P:(k + 1) * P, :])
        junk = junkp.tile([P, nsub * 512], BF16, tag="junk")
        nc.scalar.activation(out=junk, in_=wf, func=AF.Abs,
                             accum_out=asum[:, k:k + 1])
    tot = consts.tile([P, 1], F32, tag=f"{name}_tot")
    nc.vector.tensor_reduce(out=tot, in_=asum, axis=mybir.AxisListType.X,
                            op=ALU.add)
    ones128 = stage.tile([P, P], F32, tag="ones128")
    nc.vector.memset(ones128, 1.0)
    totp = ps.tile([P, 1], F32, tag="totp")
    nc.tensor.matmul(out=totp, lhsT=ones128, rhs=tot, start=True, stop=True)
    gsum = consts.tile([P, 1], F32, tag=f"{name}_gsum")
    nc.scalar.copy(out=gsum, in_=totp)
    meanclip = consts.tile([P, 1], F32, tag=f"{name}_meanclip")
    nc.vector.tensor_scalar(out=meanclip, in0=gsum, scalar1=1.0 / n_elem,
                            scalar2=1e-5, op0=ALU.mult, op1=ALU.max)
    swq = consts.tile([P, 1], F32, tag=f"{name}_swq")
    nc.vector.reciprocal(out=swq, in_=meanclip)

    wq_tiles = []
    for k in range(n_ktiles):
        wf = stage.tile([P, nsub * 512], F32, tag="stage")
        nc.gpsimd.dma_start(out=wf, in_=wT_dram[k * P:(k + 1) * P, :])
        rt = stage.tile([P, nsub * 512], F32, tag="stage_rt")
        nc.scalar.activation(out=rt, in_=wf, func=AF.Identity,
                             bias=magicb, scale=swq)
        cl = stage.tile([P, nsub * 512], F32, tag="stage_cl")
        nc.vector.tensor_scalar(out=cl, in0=rt, scalar1=MAGIC, scalar2=1.0,
                                op0=ALU.subtract, op1=ALU.min)
        wq = consts.tile([P, nsub, 512], BF16, tag=f"{name}_wq{k}")
        nc.vector.tensor_scalar(out=wq.rearrange("p a b -> p (a b)"), in0=cl,
                                scalar1=-1.0, scalar2=None, op0=ALU.max)
        wq_tiles.append(wq)
    return wq_tiles, meanclip


def build_nc_general():
    nc = bass.Bass()
    x_d = nc.dram_tensor("x", [TPC, H], F32, kind="ExternalInput")
    wupT_d = nc.dram_tensor("wupT", [H, I], F32, kind="ExternalInput")
    wdnT_d = nc.dram_tensor("wdnT", [I, H], F32, kind="ExternalInput")
    g_d = nc.dram_tensor("g", [I], F32, kind="ExternalInput")
    out_d = nc.dram_tensor("out", [TPC, H], F32, kind="ExternalOutput")

    from contextlib import ExitStack
    with ExitStack() as ctx:
        tc = ctx.enter_context(tile.TileContext(nc))
        consts = ctx.enter_context(tc.tile_pool(name="consts", bufs=1))

        ident = consts.tile([P, P], BF16)
        make_identity(nc, ident)
        magicb = consts.tile([P, 1], F32)
        nc.vector.memset(magicb, MAGIC)

        g_bc = consts.tile([P, I], F32)
        g_ap = g_d[:]
        g_bcast_ap = bass.AP(tensor=g_ap.tensor, offset=g_ap.offset,
                             ap=[[0, P]] + list(g_ap.ap))
        nc.gpsimd.dma_start(out=g_bc, in_=g_bcast_ap)

        with tc.tile_pool(name="wstage", bufs=2) as stage, \
                tc.tile_pool(name="wjunk", bufs=2) as junkp, \
                tc.tile_pool(name="wps", bufs=1, space="PSUM") as wps:
            wup_q, up_meanclip = _emit_weight_quant_g(
                nc, stage, junkp, wps, consts, wupT_d, NKH, NB, "wup", magicb)
            wdn_q, dn_meanclip = _emit_weight_quant_g(
                nc, stage, junkp, wps, consts, wdnT_d, NKI, 1, "wdn", magicb)

        k1b = consts.tile([P, 1], F32)
        nc.vector.tensor_scalar_mul(out=k1b, in0=up_meanclip,
                                    scalar1=1.0 / 127.0)
        wdk = consts.tile([P, 1], F32)
        nc.vector.tensor_scalar_mul(out=wdk, in0=dn_meanclip,
                                    scalar1=1.0 / 127.0)
        isg = consts.tile([P, 1], F32)
        nc.vector.memset(isg, 1.0 / 127.0)

        BG = 8
        xs_pool = ctx.enter_context(tc.tile_pool(name="xs", bufs=2 * BG))
        xq_pool = ctx.enter_context(tc.tile_pool(name="xqp", bufs=3))
        big = ctx.enter_context(tc.tile_pool(name="big", bufs=2))
        iup = ctx.enter_context(tc.tile_pool(name="iup", bufs=3))
        outp = ctx.enter_context(tc.tile_pool(name="outp", bufs=BG + 1))
        o2p = ctx.enter_context(tc.tile_pool(name="o2p", bufs=3))
        junkp = ctx.enter_context(tc.tile_pool(name="mjunk", bufs=1))
        small = ctx.enter_context(tc.tile_pool(name="small", bufs=3))
        batchp = ctx.enter_context(tc.tile_pool(name="batchp", bufs=2))
        ps_xT = ctx.enter_context(tc.tile_pool(name="ps_xT", bufs=1,
                                               space="PSUM"))
        ps_ih = ctx.enter_context(tc.tile_pool(name="ps_ih", bufs=1,
                                               space="PSUM"))
        ps_iuT = ctx.enter_context(tc.tile_pool(name="ps_iuT", bufs=1,
                                                space="PSUM"))
        ps_o = ctx.enter_context(tc.tile_pool(name="ps_o", bufs=1,
                                              space="PSUM"))

        IH2 = I // 2
        state = {}

        def phase_a(ib):
            xm8 = batchp.tile([P, BG], F32, tag="xm8")
            x_tiles = []
            for j in range(BG):
                r0 = (ib + j) * P
                x_sb = xs_pool.tile([P, H], F32, tag="x")
                nc.sync.dma_start(out=x_sb, in_=x_d[r0:r0 + P, :])
                x_tiles.append(x_sb)
                nc.vector.tensor_reduce(out=xm8[:, j:j + 1], in_=x_sb,
                                        axis=mybir.AxisListType.X, op=ALU.max,
                                        apply_absolute_value=True)
            t08 = batchp.tile([P, BG], F32, tag="t08")
            nc.vector.tensor_scalar_max(out=t08, in0=xm8, scalar1=1e-5)
            xr8 = batchp.tile([P, BG], F32, tag="xr8")
            nc.vector.reciprocal(out=xr8, in_=t08)
            xsc8 = batchp.tile([P, BG], F32, tag="xsc8")
            nc.vector.tensor_scalar_mul(out=xsc8, in0=xr8, scalar1=127.0)
            state[ib] = (x_tiles, t08, xsc8)

        def phase_bc(ib):
            x_tiles, t08, xsc8 = state.pop(ib)
            Sm8 = batchp.tile([P, BG], F32, tag="Sm8")
            q2g8 = batchp.tile([P, BG], F32, tag="q2g8")
            o_tiles = []

            for j in range(BG):
                x_sb = x_tiles[j]
                xq = xq_pool.tile([P, H], F32, tag="xq")
                nc.scalar.activation(out=xq, in_=x_sb, func=AF.Identity,
                                     bias=magicb, scale=xsc8[:, j:j + 1])
                ix = xq_pool.tile([P, H], BF16, tag="ix")
                nc.vector.tensor_scalar(out=ix, in0=xq, scalar1=MAGIC,
                                        scalar2=None, op0=ALU.subtract)
                xT_ps = ps_xT.tile([P, NKH, P], BF16, tag="xT")
                for k in range(NKH):
                    nc.tensor.transpose(out=xT_ps[:, k, :],
                                        in_=ix[:, k * P:(k + 1) * P],
                                        identity=ident)
                xT_sb = xq_pool.tile([P, NKH, P], BF16, tag="xTsb")
                nc.scalar.copy(out=xT_sb, in_=xT_ps)

                r_sb = big.tile([P, I], F32, tag="r")
                for h in range(2):
                    ihh = ps_ih.tile([P, IH2], F32, tag="ih")
                    for nb in range(2):
                        lo = nb * 512
                        for k in range(NKH):
                            nc.tensor.matmul(
                                out=ihh[:, lo:lo + 512],
                                lhsT=xT_sb[:, k, :],
                                rhs=wup_q[k][:, 2 * h + nb, :],
                                start=(k == 0), stop=(k == NKH - 1))
                    nc.scalar.activation(out=r_sb[:, h * IH2:(h + 1) * IH2],
                                         in_=ihh, func=AF.Relu)

                s_sb = big.tile([P, I], F32, tag="s")
                nc.gpsimd.tensor_tensor(out=s_sb, in0=r_sb, in1=r_sb,
                                        op=ALU.mult)
                sq_in = big.tile([P, I], F32, tag="sg")
                nc.vector.tensor_tensor(out=sq_in, in0=s_sb, in1=g_bc,
                                        op=ALU.mult)
                junk3 = junkp.tile([P, I], BF16, tag="junk3")
                nc.scalar.activation(out=junk3, in_=s_sb, func=AF.Square,
                                     accum_out=q2g8[:, j:j + 1])
                nc.vector.tensor_reduce(out=Sm8[:, j:j + 1], in_=sq_in,
                                        axis=mybir.AxisListType.X,
                                        op=ALU.max,
                                        apply_absolute_value=True)
                sc2 = small.tile([P, 1], F32, tag="sc2")
                nc.vector.tensor_scalar(out=sc2, in0=Sm8[:, j:j + 1],
                                        scalar1=1e-30, scalar2=isg,
                                        op0=ALU.max, op1=ALU.mult)
                dr = small.tile([P, 1], F32, tag="dr")
                nc.vector.reciprocal(out=dr, in_=sc2)
                rt = big.tile([P, I], F32, tag="rt")
                nc.vector.tensor_scalar(out=rt, in0=sq_in, scalar1=dr,
                                        scalar2=MAGIC, op0=ALU.mult,
                                        op1=ALU.add)
                iu = iup.tile([P, I], BF16, tag="iu")
                nc.vector.tensor_scalar(out=iu, in0=rt, scalar1=MAGIC,
                                        scalar2=None, op0=ALU.subtract)

                iuT_ps = ps_iuT.tile([P, NKI, P], BF16, tag="iuT")
                for k in range(NKI):
                    nc.tensor.transpose(out=iuT_ps[:, k, :],
                                        in_=iu[:, k * P:(k + 1) * P],
                                        identity=ident)
                iuT_sb = iup.tile([P, NKI, P], BF16, tag="iuTsb")
                nc.scalar.copy(out=iuT_sb, in_=iuT_ps)

                o_ps = ps_o.tile([P, H], F32, tag="o")
                for k in range(NKI):
                    nc.tensor.matmul(out=o_ps, lhsT=iuT_sb[:, k, :],
                                     rhs=wdn_q[k][:, 0, :],
                                     start=(k == 0), stop=(k == NKI - 1))
                o_sb = outp.tile([P, H], F32, tag="osb")
                nc.scalar.copy(out=o_sb, in_=o_ps)
                o_tiles.append(o_sb)

            scc8 = batchp.tile([P, BG], F32, tag="scc8")
            nc.vector.tensor_scalar_max(out=scc8, in0=Sm8, scalar1=1e-30)
            ga8 = batchp.tile([P, BG], F32, tag="ga8")
            nc.vector.tensor_scalar_mul(out=ga8, in0=t08, scalar1=k1b)
            al8 = batchp.tile([P, BG], F32, tag="al8")
            nc.vector.tensor_tensor(out=al8, in0=ga8, in1=ga8, op=ALU.mult)
            m18 = batchp.tile([P, BG], F32, tag="m18")
            nc.vector.tensor_tensor(out=m18, in0=al8, in1=scc8, op=ALU.mult)
            v18 = batchp.tile([P, BG], F32, tag="v18")
            Ve8 = batchp.tile([P, BG], F32, tag="Ve8")
            al28 = batchp.tile([P, BG], F32, tag="al28")
            nc.vector.tensor_tensor(out=al28, in0=al8, in1=al8, op=ALU.mult)
            nc.vector.tensor_tensor(out=v18, in0=al28, in1=q2g8, op=ALU.mult)
            nc.vector.tensor_scalar(out=Ve8, in0=v18, scalar1=1.0 / I,
                                    scalar2=EPS, op0=ALU.mult, op1=ALU.add)
            sq8 = batchp.tile([P, BG], F32, tag="sq8")
            nc.scalar.activation(out=sq8, in_=Ve8, func=AF.Sqrt)
            cr8 = batchp.tile([P, BG], F32, tag="cr8")
            nc.vector.reciprocal(out=cr8, in_=sq8)
            h18 = batchp.tile([P, BG], F32, tag="h18")
            nc.vector.tensor_tensor(out=h18, in0=cr8, in1=cr8, op=ALU.mult)
            h28 = batchp.tile([P, BG], F32, tag="h28")
            nc.vector.tensor_tensor(out=h28, in0=h18, in1=Ve8, op=ALU.mult)
            h38 = batchp.tile([P, BG], F32, tag="h38")
            nc.vector.tensor_scalar(out=h38, in0=h28, scalar1=-0.5,
                                    scalar2=1.5, op0=ALU.mult, op1=ALU.add)
            c8 = batchp.tile([P, BG], F32, tag="c8")
            nc.vector.tensor_tensor(out=c8, in0=cr8, in1=h38, op=ALU.mult)
            mu8 = batchp.tile([P, BG], F32, tag="mu8")
            nc.vector.tensor_tensor(out=mu8, in0=c8, in1=m18, op=ALU.mult)
            b8 = batchp.tile([P, BG], F32, tag="b8")
            nc.vector.tensor_scalar(out=b8, in0=mu8, scalar1=1e-5,
                                    scalar2=wdk, op0=ALU.max, op1=ALU.mult)

            for j in range(BG):
                r0 = (ib + j) * P
                o2 = o2p.tile([P, H], F32, tag="o2")
                nc.vector.tensor_scalar_mul(out=o2, in0=o_tiles[j],
                                            scalar1=b8[:, j:j + 1])
                nc.sync.dma_start(out=out_d[r0:r0 + P, :], in_=o2)

        phase_a(0)
        for ib in range(0, NT, BG):
            if ib + BG < NT:
                phase_a(ib + BG)
            phase_bc(ib)

    _split_sync_waits(nc)
    return nc


_NC_CACHE = {}


def kernel(x, w_up, w_down, g):
    global LAST_RESULT
    x = np.ascontiguousarray(x, dtype=np.float32)
    w_up = np.ascontiguousarray(w_up, dtype=np.float32)
    w_down = np.ascontiguousarray(w_down, dtype=np.float32)
    g = np.ascontiguousarray(g, dtype=np.float32)

    if abs(float(g[0])) < 1e-30 and np.all(g == g[0]):
        return np.zeros_like(x)

    general = not bool(np.all(g == g[0]))
    key = ("gen" if general else "const")
    if key not in _NC_CACHE:
        _NC_CACHE[key] = (build_nc_general() if general
                          else build_nc_const())
    nc = _NC_CACHE[key]

    xt = x.reshape(TOK, H)
    wupT = np.ascontiguousarray(w_up.T)    # [H, I]
    wdnT = np.ascontiguousarray(w_down.T)  # [I, H]
    in_maps = [
        {"x": xt[c * TPC:(c + 1) * TPC], "wupT": wupT, "wdnT": wdnT, "g": g}
        for c in range(N_CORES)
    ]
    res = run_bass_kernel_spmd(
        nc, in_maps, list(range(N_CORES)),
        trace=bool(os.environ.get("BASS_TRACE")),
    )
    LAST_RESULT = res
    out = np.concatenate([res.results[c]["out"] for c in range(N_CORES)],
                         axis=0)
    return out.reshape(B, S, H)
